# revision 48
# baseline (speedup 1.0000x reference)
"""Trainium2 Bass kernel: tridiagonal solve A(alpha) x = f, N = 4M, f32.

A is strictly diagonally dominant (b = 1+alpha^3 >= 1, sub = alpha^2 <= 0.09,
super = alpha^2 + 2 alpha <= 0.69 for alpha in [0, 0.3)).  All Thomas-algorithm
recurrences therefore forget their initial conditions at a geometric rate
(forward: |a/m| <= 0.097 per step, backward: |cp| <= 0.74 per step), so the
global sequential solve can be replaced by fully independent overlapping
windows: each of 8 cores x 128 lanes owns a contiguous 4096-element chunk and
computes it exactly (to fp32) using a 16-element forward warmup halo and a
64-element backward warmup halo.  No reduced system, no collectives.

The affine recurrences (forward-eliminated rhs dp, and back-substitution)
run on the DVE tensor_tensor_scan instruction (state = d0*state + d1 per
partition along the free dim; the back-substitution scan uses negative-stride
APs to run in reverse).  The nonlinear pivot recurrence, normalized as
g_t = 1/(1 - rho_t g_{t-1}) with rho_t = su_t/(b_t b_{t-1}) in [0, 0.075],
is solved by Jacobi sweeps where one custom 8-stage DVE op applies the whole
map exactly: the degree-7 truncation (1+t)(1+t^2)(1+t^4) of 1/(1-t) is
fp32-exact for t <= 0.075.  1/b = chi(alpha) is likewise an fp32-exact cubic
series.  No reciprocal/divide instructions anywhere.

Measured on trn2 (8 cores): ~87.5 us NEFF exec, absmax/scale error ~8.5e-7
(the fp32 reference itself is ~1.6e-7 from float64).
"""

import contextlib

import numpy as np

import concourse.bacc as bacc
import concourse.bass as bass
import concourse.mybir as mybir
import concourse.tile as tile

N = 4_194_304
NCORES = 8
P = 128
D = N // (NCORES * P)  # 4096 elements per lane
HF = 16   # forward warmup halo
HB = 64   # backward warmup halo
K_SWEEPS = 3  # fixed-point sweeps for the pivot sequence m (v1 algo)
FINAL_ACC = True  # accurate (2-op) final reciprocal vs fast (1-op)  (v1 algo)
K_G = 4   # total g-updates (init + steps) in the g-form algo
F32 = mybir.dt.float32
ALU = mybir.AluOpType
ACTF = mybir.ActivationFunctionType

# ---------------------------------------------------------------------------
# Custom DVE ops for the g-form pipeline.
#
# The normalized pivot recurrence is mu_t = 1 - rho_t / mu_{t-1} with
# rho_t = su_t / (b_t b_{t-1}) in [0, 0.075].  Iterating directly on
# g := 1/mu:  g_t = 1/(1 - rho_t g_{t-1}).  Since t = rho*g <= 0.075, the
# degree-7 truncation (1+t)(1+t^2)(1+t^4) of 1/(1-t) is exact to fp32
# (t^8 < 1e-9), so one 8-stage custom DVE op implements a whole sweep with
# no reciprocal anywhere.  chi(x) = 1/(1+x^3) by the same argument
# (x^3 <= 0.027, quartic-term error < 6e-7).
# ---------------------------------------------------------------------------
import numpy as _np
from concourse import dve_ops as _dvo
from concourse.dve_spec import Spec as _Spec, Src0 as _S0, Src1 as _S1, One as _One
from concourse.dve_spec import lower as _dve_lower, _has_src1
from concourse.dve_table_gen import dve_ver_for as _dve_ver_for
from concourse.dve_uop import DveOpSpec as _DveOpSpec


def _register_dve_op(name, spec, subdim=False):
    existing = {op.name: op for op in _dvo.OPS}
    if name in existing:
        return existing[name]
    row = max(_dvo._SUB_OPCODE_FOR_NAME.values()) + 1
    assert row < 0x20
    shas = {}
    for ver in ("v3", "v4"):
        compiled = _DveOpSpec(
            name=name, opcode=row, uops=_dve_lower(spec, ver=ver),
            rd1_en=_has_src1(spec),
        )
        shas[ver] = compiled.sha(ver)
        _dvo._COMPILE_CACHE[(name, ver)] = compiled
    op = _dvo.DveOp(name, spec, subdim=subdim, uops_sha=shas)
    _dvo.OPS.append(op)
    _dvo._SUB_OPCODE_FOR_NAME[name] = row
    _dvo.CUSTOM_DVE_SPECS[name] = spec
    return op


def _series_chi(c):
    # 1 - c + c^2 - c^3 = 1 - c*(1 - c*(1 - c))
    return _One - c * (_One - c * (_One - c))


def _ref_chi(in0, in1, c0, c1, c2):
    x = in0.astype(_np.float32)
    c = x * x * x
    one = _np.float32(1.0)
    return one - c * (one - c * (one - c))


def _telescope(t):
    # (1+t)(1+t^2)(1+t^4) = sum_{j=0..7} t^j  ~= 1/(1-t) for |t| << 1
    t2 = t * t
    t4 = t2 * t2
    return ((_One + t) * (_One + t2)) * (_One + t4)


def _ref_gs_init(in0, in1, c0, c1, c2):
    t = in0.astype(_np.float32)
    t2 = t * t
    t4 = t2 * t2
    return ((1 + t) * (1 + t2)) * (1 + t4)


def _ref_gs_step(in0, in1, c0, c1, c2):
    t = (in0.astype(_np.float32) * in1.astype(_np.float32)).astype(_np.float32)
    t2 = t * t
    t4 = t2 * t2
    return ((1 + t) * (1 + t2)) * (1 + t4)


def _ref_negmul(in0, in1, c0, c1, c2):
    return -(in0.astype(_np.float32) * in1.astype(_np.float32))


_sq0 = _S0 * _S0
OP_CHI = _register_dve_op(
    "TRIDIAG_CHI", _Spec(body=_series_chi(_sq0 * _S0), reference=_ref_chi)
)
OP_GS_INIT = _register_dve_op(
    "TRIDIAG_GS_INIT", _Spec(body=_telescope(_S0), reference=_ref_gs_init)
)
OP_GS_STEP = _register_dve_op(
    "TRIDIAG_GS_STEP", _Spec(body=_telescope(_S0 * _S1), reference=_ref_gs_step)
)
OP_NEGMUL = _register_dve_op(
    "TRIDIAG_NEGMUL", _Spec(body=-(_S0 * _S1), reference=_ref_negmul)
)


def _ref_subone_mul(in0, in1, c0, c1, c2):
    return (in0.astype(_np.float32) - _np.float32(1.0)) * in1.astype(_np.float32)


def _ref_onesub_mul(in0, in1, c0, c1, c2):
    return (_np.float32(1.0) - in0.astype(_np.float32)) * in1.astype(_np.float32)


OP_SUBONE_MUL = _register_dve_op(
    "TRIDIAG_SUBONE_MUL", _Spec(body=(_S0 - _One) * _S1, reference=_ref_subone_mul)
)
OP_ONESUB_MUL = _register_dve_op(
    "TRIDIAG_ONESUB_MUL", _Spec(body=(_One - _S0) * _S1, reference=_ref_onesub_mul)
)


def _np_chi(x, terms=3):
    c = (x * x * x).astype(_np.float32)
    one = _np.float32(1.0)
    if terms == 3:
        return one - c * (one - c * (one - c))
    return one - c * (one - c)


def _ref_phi(in0, in1, c0, c1, c2):
    x = in0.astype(_np.float32)
    return (x * x) * _np_chi(x)


def _ref_psi(in0, in1, c0, c1, c2):
    x = in0.astype(_np.float32)
    return (x * (x + _np.float32(2.0))) * _np_chi(x, terms=2)


def _ref_rchi(in0, in1, c0, c1, c2):
    x = in0.astype(_np.float32)
    return _np_chi(x) * in1.astype(_np.float32)


_TWO = _One + _One
OP_PHI = _register_dve_op(
    "TRIDIAG_PHI", _Spec(body=_series_chi(_sq0 * _S0) * _sq0, reference=_ref_phi)
)
_c_psi = _sq0 * _S0
OP_PSI = _register_dve_op(
    "TRIDIAG_PSI",
    _Spec(
        body=(_S0 * (_S0 + _TWO)) * (_One - _c_psi * (_One - _c_psi)),
        reference=_ref_psi,
    ),
)
OP_RCHI = _register_dve_op(
    "TRIDIAG_RCHI", _Spec(body=_series_chi(_sq0 * _S0) * _S1, reference=_ref_rchi)
)


def _ref_gstep1(in0, in1, c0, c1, c2):
    t = (in0.astype(_np.float32) * (1 + in1.astype(_np.float32))).astype(_np.float32)
    t2 = t * t
    return (1 + t) * (1 + t2)


_t_g1 = _S0 * (_One + _S1)
_t2_g1 = _t_g1 * _t_g1
OP_GSTEP1 = _register_dve_op(
    "TRIDIAG_GSTEP1",
    _Spec(body=(_One + _t_g1) * (_One + _t2_g1), reference=_ref_gstep1),
)


def emit_core(
    tc, alpha_in, f_in, x_out, D=D, K=K_SWEEPS, HF=HF, HB=HB, final_acc=True
):
    """Emit one core's program.

    alpha_in: dram handle [P*D + HF + HB + 2]  (alpha padded: lane p uses
              [p*D, p*D + T + 2), covering global rows g-1, g, g+1 for its
              window rows g in [p*D - HF, p*D + D + HB))
    f_in:     dram handle [P*D + HF + HB]
    x_out:    dram handle [P*D]
    """
    nc = tc.nc
    T = HF + D + HB
    TA = T + 2
    with contextlib.ExitStack() as ctx:
        pool = ctx.enter_context(tc.tile_pool(name="w", bufs=1))
        t_alpha = pool.tile([P, TA], F32, tag="alpha")  # later reused: q
        t_f = pool.tile([P, TA], F32, tag="f")          # later: B, in place
        t_sq = pool.tile([P, TA], F32, tag="sq")        # later: A, in place
        t_um = pool.tile([P, TA], F32, tag="um")        # (alpha+1)^2 - 1 via ACT
        t_ncp = pool.tile([P, TA], F32, tag="ncp")      # negcp (GPSIMD)
        t_cube = pool.tile([P, TA], F32, tag="cube")    # later: x
        t_su = pool.tile([P, TA], F32, tag="su")        # later: dp
        t_m = pool.tile([P, TA], F32, tag="m")
        t_r = pool.tile([P, TA], F32, tag="r")

        nc.sync.dma_start(t_alpha[:], bass.AP(alpha_in, 0, [[D, P], [1, TA]]))
        nc.sync.dma_start(t_f[:, 0:T], bass.AP(f_in, 0, [[D, P], [1, T]]))

        # ACT: sq = alpha^2 ; um = (alpha+1)^2 - 1 = alpha^2 + 2 alpha
        nc.scalar.activation(t_sq[:], t_alpha[:], ACTF.Square)
        nc.scalar.activation(t_um[:], t_alpha[:], ACTF.Square, bias=1.0)
        nc.scalar.activation(t_um[:], t_um[:], ACTF.Identity, bias=-1.0)
        # cube[k] = alpha[k]^3
        nc.vector.tensor_tensor(t_cube[:], t_sq[:], t_alpha[:], ALU.mult)
        # su[t] = s_t * u_{t-1} = sq[t] * um[t+1]   (row t: g = lane_base - HF + t)
        nc.vector.tensor_tensor(
            t_su[:, 0:T], t_sq[:, 0:T], t_um[:, 1 : T + 1], ALU.mult
        )
        # m^0 = b = 1 + cube[t+1]   (tensor_scalar: 2x perf mode)
        nc.vector.tensor_scalar(t_m[:, 0:T], t_cube[:, 1 : T + 1], 1.0, None, ALU.add)
        # fixed point:  m[t] = b[t] - su[t] * r[t-1],  r = 1/m
        q = t_alpha  # alpha dead after cube
        for k in range(K):
            nc.vector.reciprocal_approx_fast(out=t_r[:, 0:T], in_=t_m[:, 0:T])
            nc.vector.tensor_tensor(
                q[:, 0 : T - 1], t_su[:, 1:T], t_r[:, 0 : T - 1], ALU.mult
            )
            nc.vector.scalar_tensor_tensor(
                t_m[:, 1:T], t_cube[:, 2 : T + 1], 1.0, q[:, 0 : T - 1],
                ALU.add, ALU.subtract,
            )
        if final_acc:
            nc.vector.reciprocal_approx_accurate(
                out=t_r[:, 0:T], in_=t_m[:, 0:T], scratch=q[:, 0:T]
            )
        else:
            nc.vector.reciprocal_approx_fast(out=t_r[:, 0:T], in_=t_m[:, 0:T])

        # rn = -r  (tensor_scalar: 2x perf mode)
        rn = t_m  # m dead after final reciprocal
        nc.vector.tensor_scalar(rn[:, 0:T], t_r[:, 0:T], -1.0, None, ALU.mult)
        # negcp = -u*r = um[t+2]*rn   (GPSIMD, overlaps the dp scan)
        nc.gpsimd.tensor_tensor(
            t_ncp[:, 0:T], t_um[:, 2 : T + 2], rn[:, 0:T], ALU.mult
        )
        # A = -s*r = sq*rn ; B = f*r
        nc.vector.tensor_tensor(t_sq[:, 0:T], t_sq[:, 0:T], rn[:, 0:T], ALU.mult)
        nc.vector.tensor_tensor(t_f[:, 0:T], t_f[:, 0:T], t_r[:, 0:T], ALU.mult)
        # dp scan: dp[t] = A[t]*dp[t-1] + B[t]
        nc.vector.tensor_tensor_scan(
            t_su[:, 0:T], t_sq[:, 0:T], t_f[:, 0:T], 0.0, ALU.mult, ALU.add
        )
        # backward scan (reversed): x[t] = negcp[t]*x[t+1] + dp[t]
        nc.vector.tensor_tensor_scan(
            t_cube[:, 0:T][:, ::-1],
            t_ncp[:, 0:T][:, ::-1],
            t_su[:, 0:T][:, ::-1],
            0.0,
            ALU.mult,
            ALU.add,
        )
        nc.sync.dma_start(
            bass.AP(x_out, 0, [[D, P], [1, D]]), t_cube[:, HF : HF + D]
        )


def emit_core_g(tc, alpha_in, f_in, x_out, D=D, K=K_G, HF=HF, HB=HB):
    """g-form pipeline: custom-DVE series ops, no reciprocal instructions.

    rho_t = su_t/(b_t b_{t-1}) = (sq[t] X[t]) * (umm[t+1] X[t+1]),
    g = 1/mu via fixed point  g <- (1+t)(1+t^2)(1+t^4), t = rho*g_prev,
    r_t = 1/m_t = X[t+1] * g_t.
    """
    nc = tc.nc
    T = HF + D + HB
    TA = T + 2
    with contextlib.ExitStack() as ctx:
        pool = ctx.enter_context(tc.tile_pool(name="w", bufs=1))
        t_alpha = pool.tile([P, TA], F32, tag="alpha")
        t_f = pool.tile([P, TA], F32, tag="f")      # -> B in place
        t_sq = pool.tile([P, TA], F32, tag="sq")    # -> A in place
        t_u1 = pool.tile([P, TA], F32, tag="u1")    # (alpha+1)^2; -> negcp
        t_pp = pool.tile([P, TA], F32, tag="pp")    # -> rho in place -> x (scan2)
        t_qq = pool.tile([P, TA], F32, tag="qq")    # -> dp (scan1)
        t_g = pool.tile([P, TA], F32, tag="g")
        t_r = pool.tile([P, TA], F32, tag="r")

        # Head is column-chunked so ACT/DVE start on the first part of alpha
        # while the rest is still in flight.
        ncol = NCOL_HEAD
        if GEO_HEAD:
            w0 = TA // (2 ** ncol - 1)
            edges = [0]
            for j in range(ncol - 1):
                edges.append(edges[-1] + w0 * (2 ** j))
            edges.append(TA)
        else:
            edges = [0] + [TA * (j + 1) // ncol for j in range(ncol)]
        for j in range(ncol):
            lo, hi = edges[j], edges[j + 1]
            dma_eng = nc.sync if j % 2 == 0 else nc.scalar
            dma_eng.dma_start(
                t_alpha[:, lo:hi], bass.AP(alpha_in, lo, [[D, P], [1, hi - lo]])
            )
            # ACT: sq = alpha^2 ; u1 = (alpha+1)^2 = alpha^2 + 2 alpha + 1
            nc.scalar.activation(t_sq[:, lo:hi], t_alpha[:, lo:hi], ACTF.Square)
            nc.scalar.activation(
                t_u1[:, lo:hi], t_alpha[:, lo:hi], ACTF.Square, bias=1.0
            )
            if HEAD_V2:
                # PP = phi(alpha) = alpha^2 chi(alpha); QQ = psi(alpha)
                nc.vector._custom_dve(
                    OP_PHI, out=t_pp[:, lo:hi], in0=t_alpha[:, lo:hi]
                )
                nc.vector._custom_dve(
                    OP_PSI, out=t_qq[:, lo:hi], in0=t_alpha[:, lo:hi]
                )
            else:
                nc.vector._custom_dve(
                    OP_CHI, out=t_r[:, lo:hi], in0=t_alpha[:, lo:hi]
                )
                nc.vector.tensor_tensor(
                    t_pp[:, lo:hi], t_sq[:, lo:hi], t_r[:, lo:hi], ALU.mult
                )
                nc.vector._custom_dve(
                    OP_SUBONE_MUL, out=t_qq[:, lo:hi], in0=t_u1[:, lo:hi],
                    in1=t_r[:, lo:hi],
                )
        # rho_t = PP[t]*QQ[t+1]
        rho = t_pp[:, 0:T]
        nc.vector.tensor_tensor(rho, t_pp[:, 0:T], t_qq[:, 1 : T + 1], ALU.mult)
        # g fixed point.  Init g0 = 1 + rho (tensor_scalar: 2x perf mode);
        # each GS_STEP then applies the exact truncated map in place
        # (the in-place shifted read sees the previous sweep's values:
        # position t-1 is read two cycles before its new value lands).
        if GS_TS_INIT == "gstep1":
            # g[0] = 1 + rho[0];  g[1:] = telescope3(rho*(1+rho@-1))
            # (two map applications in one 6-stage op)
            nc.vector.tensor_scalar(
                t_g[:, 0:1], t_pp[:, 0:1], 1.0, None, ALU.add
            )
            nc.vector._custom_dve(
                OP_GSTEP1, out=t_g[:, 1:T], in0=t_pp[:, 1:T],
                in1=t_pp[:, 0 : T - 1],
            )
            nsteps = K - 2
        elif GS_TS_INIT:
            nc.vector.tensor_scalar(t_g[:, 0:T], rho, 1.0, None, ALU.add)
            nsteps = K - 1
        else:
            nc.vector._custom_dve(OP_GS_INIT, out=t_g[:, 0:T], in0=rho)
            nsteps = K - 1
        for _ in range(nsteps):
            nc.vector._custom_dve(
                OP_GS_STEP, out=t_g[:, 1:T], in0=t_pp[:, 1:T],
                in1=t_g[:, 0 : T - 1],
            )
        # f arrives late: its only consumer (B) is ~60us into the pipeline,
        # so don't let it compete with the alpha load at kernel start.
        nc.sync.dma_start(t_f[:, 0:T], bass.AP(f_in, 0, [[D, P], [1, T]]))
        # r = chi(alpha[t+1])*g
        if HEAD_V2:
            nc.vector._custom_dve(
                OP_RCHI, out=t_r[:, 0:T], in0=t_alpha[:, 1 : T + 1],
                in1=t_g[:, 0:T],
            )
        else:
            # t_r holds X = chi(alpha); write r over it (write trails read)
            nc.vector.tensor_tensor(
                t_r[:, 0:T], t_r[:, 1 : T + 1], t_g[:, 0:T], ALU.mult
            )
        # A = -sq*r (in place) ; B = f*r (in place) ; negcp = (1-u1[t+2])*r
        nc.vector._custom_dve(
            OP_NEGMUL, out=t_sq[:, 0:T], in0=t_sq[:, 0:T], in1=t_r[:, 0:T]
        )
        nc.vector.tensor_tensor(t_f[:, 0:T], t_f[:, 0:T], t_r[:, 0:T], ALU.mult)
        nc.vector._custom_dve(
            OP_ONESUB_MUL, out=t_u1[:, 0:T], in0=t_u1[:, 2 : T + 2], in1=t_r[:, 0:T]
        )
        # dp scan then reversed back-substitution scan.  The reversed scan is
        # split into column chunks (high chunk first) chained through
        # `initial`, so each chunk's output DMA overlaps the next chunk's scan.
        nc.vector.tensor_tensor_scan(
            t_qq[:, 0:T], t_sq[:, 0:T], t_f[:, 0:T], 0.0, ALU.mult, ALU.add
        )
        nchunk = SCAN2_CHUNKS
        cuts = [0] + [HF + (D * (j + 1)) // nchunk for j in range(nchunk - 1)] + [T]
        for ci in range(len(cuts) - 2, -1, -1):
            lo, hi = cuts[ci], cuts[ci + 1]
            init = 0.0 if hi == T else t_pp[:, hi : hi + 1]
            nc.vector.tensor_tensor_scan(
                t_pp[:, lo:hi][:, ::-1],
                t_u1[:, lo:hi][:, ::-1],
                t_qq[:, lo:hi][:, ::-1],
                init,
                ALU.mult,
                ALU.add,
            )
            slo, shi = max(lo, HF), min(hi, HF + D)
            nc.sync.dma_start(
                bass.AP(x_out, slo - HF, [[D, P], [1, shi - slo]]),
                t_pp[:, slo:shi],
            )


ALGO = "g"  # "g" (custom-op pipeline) or "v1" (stock-op pipeline)
HEAD_V2 = True  # PHI/PSI/RCHI fused head vs CHI+tt head
NCOL_HEAD = 4
SCAN2_CHUNKS = 4
GEO_HEAD = True
GS_TS_INIT = "gstep1"  # "gstep1" | True (ts 1+rho) | False (GS_INIT custom)


def build_nc(D=D, K=K_SWEEPS, HF=HF, HB=HB, ncores=NCORES, final_acc=True):
    C = P * D
    nc = bacc.Bacc(
        "TRN2", target_bir_lowering=False, debug=False, num_devices=ncores
    )
    alpha_in = nc.dram_tensor("alpha_in", [C + HF + HB + 2], F32, kind="ExternalInput")
    f_in = nc.dram_tensor("f_in", [C + HF + HB], F32, kind="ExternalInput")
    x_out = nc.dram_tensor("x_out", [C], F32, kind="ExternalOutput")
    with tile.TileContext(nc) as tc:
        if ALGO == "g":
            emit_core_g(tc, alpha_in, f_in, x_out, D=D, K=K_G, HF=HF, HB=HB)
        else:
            emit_core(tc, alpha_in, f_in, x_out, D=D, K=K, HF=HF, HB=HB,
                      final_acc=final_acc)
    nc.compile()
    return nc


def shard_inputs(alpha, f, D=D, HF=HF, HB=HB, ncores=NCORES):
    C = P * D
    n = ncores * C
    alpha_pad = np.zeros(n + HF + HB + 2, dtype=np.float32)
    alpha_pad[HF + 1 : HF + 1 + n] = alpha
    f_pad = np.zeros(n + HF + HB, dtype=np.float32)
    f_pad[HF : HF + n] = f
    in_maps = []
    for c in range(ncores):
        in_maps.append(
            {
                "alpha_in": np.ascontiguousarray(alpha_pad[c * C : c * C + C + HF + HB + 2]),
                "f_in": np.ascontiguousarray(f_pad[c * C : c * C + C + HF + HB]),
            }
        )
    return in_maps


_NC_CACHE = {}


def kernel(alpha: np.ndarray, f: np.ndarray, trace: bool = False, **run_kwargs):
    from concourse import bass_utils

    alpha = np.asarray(alpha, dtype=np.float32)
    f = np.asarray(f, dtype=np.float32)
    assert alpha.shape == (N,) and f.shape == (N,)
    key = (D, K_SWEEPS, HF, HB, FINAL_ACC, ALGO, K_G, HEAD_V2, NCOL_HEAD, SCAN2_CHUNKS, GS_TS_INIT, GEO_HEAD)
    if key not in _NC_CACHE:
        _NC_CACHE[key] = build_nc(K=K_SWEEPS, final_acc=FINAL_ACC)
    nc = _NC_CACHE[key]
    in_maps = shard_inputs(alpha, f)
    res = bass_utils.run_bass_kernel_spmd(
        nc, in_maps, core_ids=list(range(NCORES)), trace=trace, **run_kwargs
    )
    out = np.concatenate([res.results[c]["x_out"] for c in range(NCORES)])
    if trace:
        kernel.last_results = res
    return out


# revision 49
# speedup vs baseline: 1.0207x; 1.0207x over previous
"""Trainium2 Bass kernel: tridiagonal solve A(alpha) x = f, N = 4M, f32.

A is strictly diagonally dominant (b = 1+alpha^3 >= 1, sub = alpha^2 <= 0.09,
super = alpha^2 + 2 alpha <= 0.69 for alpha in [0, 0.3)).  All Thomas-algorithm
recurrences therefore forget their initial conditions at a geometric rate
(forward: |a/m| <= 0.097 per step, backward: |cp| <= 0.74 per step), so the
global sequential solve can be replaced by fully independent overlapping
windows: each of 8 cores x 128 lanes owns a contiguous 4096-element chunk and
computes it exactly (to fp32) using a 16-element forward warmup halo and a
64-element backward warmup halo.  No reduced system, no collectives.

The affine recurrences (forward-eliminated rhs dp, and back-substitution)
run on the DVE tensor_tensor_scan instruction (state = d0*state + d1 per
partition along the free dim; the back-substitution scan uses negative-stride
APs to run in reverse).  The nonlinear pivot recurrence, normalized as
g_t = 1/(1 - rho_t g_{t-1}) with rho_t = su_t/(b_t b_{t-1}) in [0, 0.075],
is solved by Jacobi sweeps where one custom 8-stage DVE op applies the whole
map exactly: the degree-7 truncation (1+t)(1+t^2)(1+t^4) of 1/(1-t) is
fp32-exact for t <= 0.075.  1/b = chi(alpha) is likewise an fp32-exact cubic
series.  No reciprocal/divide instructions anywhere.

Measured on trn2 (8 cores): ~87.5 us NEFF exec, absmax/scale error ~8.5e-7
(the fp32 reference itself is ~1.6e-7 from float64).
"""

import contextlib

import numpy as np

import concourse.bacc as bacc
import concourse.bass as bass
import concourse.mybir as mybir
import concourse.tile as tile

N = 4_194_304
NCORES = 8
P = 128
D = N // (NCORES * P)  # 4096 elements per lane
HF = 16   # forward warmup halo
HB = 64   # backward warmup halo
K_SWEEPS = 3  # fixed-point sweeps for the pivot sequence m (v1 algo)
FINAL_ACC = True  # accurate (2-op) final reciprocal vs fast (1-op)  (v1 algo)
K_G = 4   # total g-updates (init + steps) in the g-form algo
F32 = mybir.dt.float32
ALU = mybir.AluOpType
ACTF = mybir.ActivationFunctionType

# ---------------------------------------------------------------------------
# Custom DVE ops for the g-form pipeline.
#
# The normalized pivot recurrence is mu_t = 1 - rho_t / mu_{t-1} with
# rho_t = su_t / (b_t b_{t-1}) in [0, 0.075].  Iterating directly on
# g := 1/mu:  g_t = 1/(1 - rho_t g_{t-1}).  Since t = rho*g <= 0.075, the
# degree-7 truncation (1+t)(1+t^2)(1+t^4) of 1/(1-t) is exact to fp32
# (t^8 < 1e-9), so one 8-stage custom DVE op implements a whole sweep with
# no reciprocal anywhere.  chi(x) = 1/(1+x^3) by the same argument
# (x^3 <= 0.027, quartic-term error < 6e-7).
# ---------------------------------------------------------------------------
import numpy as _np
from concourse import dve_ops as _dvo
from concourse.dve_spec import Spec as _Spec, Src0 as _S0, Src1 as _S1, One as _One
from concourse.dve_spec import lower as _dve_lower, _has_src1
from concourse.dve_table_gen import dve_ver_for as _dve_ver_for
from concourse.dve_uop import DveOpSpec as _DveOpSpec


def _register_dve_op(name, spec, subdim=False):
    existing = {op.name: op for op in _dvo.OPS}
    if name in existing:
        return existing[name]
    row = max(_dvo._SUB_OPCODE_FOR_NAME.values()) + 1
    assert row < 0x20
    shas = {}
    for ver in ("v3", "v4"):
        compiled = _DveOpSpec(
            name=name, opcode=row, uops=_dve_lower(spec, ver=ver),
            rd1_en=_has_src1(spec),
        )
        shas[ver] = compiled.sha(ver)
        _dvo._COMPILE_CACHE[(name, ver)] = compiled
    op = _dvo.DveOp(name, spec, subdim=subdim, uops_sha=shas)
    _dvo.OPS.append(op)
    _dvo._SUB_OPCODE_FOR_NAME[name] = row
    _dvo.CUSTOM_DVE_SPECS[name] = spec
    return op


def _series_chi(c):
    # 1 - c + c^2 - c^3 = 1 - c*(1 - c*(1 - c))
    return _One - c * (_One - c * (_One - c))


def _ref_chi(in0, in1, c0, c1, c2):
    x = in0.astype(_np.float32)
    c = x * x * x
    one = _np.float32(1.0)
    return one - c * (one - c * (one - c))


def _telescope(t):
    # (1+t)(1+t^2)(1+t^4) = sum_{j=0..7} t^j  ~= 1/(1-t) for |t| << 1
    t2 = t * t
    t4 = t2 * t2
    return ((_One + t) * (_One + t2)) * (_One + t4)


def _ref_gs_init(in0, in1, c0, c1, c2):
    t = in0.astype(_np.float32)
    t2 = t * t
    t4 = t2 * t2
    return ((1 + t) * (1 + t2)) * (1 + t4)


def _ref_gs_step(in0, in1, c0, c1, c2):
    t = (in0.astype(_np.float32) * in1.astype(_np.float32)).astype(_np.float32)
    t2 = t * t
    t4 = t2 * t2
    return ((1 + t) * (1 + t2)) * (1 + t4)


def _ref_negmul(in0, in1, c0, c1, c2):
    return -(in0.astype(_np.float32) * in1.astype(_np.float32))


_sq0 = _S0 * _S0
OP_CHI = _register_dve_op(
    "TRIDIAG_CHI", _Spec(body=_series_chi(_sq0 * _S0), reference=_ref_chi)
)
OP_GS_INIT = _register_dve_op(
    "TRIDIAG_GS_INIT", _Spec(body=_telescope(_S0), reference=_ref_gs_init)
)
OP_GS_STEP = _register_dve_op(
    "TRIDIAG_GS_STEP", _Spec(body=_telescope(_S0 * _S1), reference=_ref_gs_step)
)
OP_NEGMUL = _register_dve_op(
    "TRIDIAG_NEGMUL", _Spec(body=-(_S0 * _S1), reference=_ref_negmul)
)


def _ref_subone_mul(in0, in1, c0, c1, c2):
    return (in0.astype(_np.float32) - _np.float32(1.0)) * in1.astype(_np.float32)


def _ref_onesub_mul(in0, in1, c0, c1, c2):
    return (_np.float32(1.0) - in0.astype(_np.float32)) * in1.astype(_np.float32)


OP_SUBONE_MUL = _register_dve_op(
    "TRIDIAG_SUBONE_MUL", _Spec(body=(_S0 - _One) * _S1, reference=_ref_subone_mul)
)
OP_ONESUB_MUL = _register_dve_op(
    "TRIDIAG_ONESUB_MUL", _Spec(body=(_One - _S0) * _S1, reference=_ref_onesub_mul)
)


def _np_chi(x, terms=3):
    c = (x * x * x).astype(_np.float32)
    one = _np.float32(1.0)
    if terms == 3:
        return one - c * (one - c * (one - c))
    return one - c * (one - c)


def _ref_phi(in0, in1, c0, c1, c2):
    x = in0.astype(_np.float32)
    return (x * x) * _np_chi(x)


def _ref_psi(in0, in1, c0, c1, c2):
    x = in0.astype(_np.float32)
    return (x * (x + _np.float32(2.0))) * _np_chi(x, terms=2)


def _ref_rchi(in0, in1, c0, c1, c2):
    x = in0.astype(_np.float32)
    return _np_chi(x) * in1.astype(_np.float32)


_TWO = _One + _One
OP_PHI = _register_dve_op(
    "TRIDIAG_PHI", _Spec(body=_series_chi(_sq0 * _S0) * _sq0, reference=_ref_phi)
)
_c_psi = _sq0 * _S0
OP_PSI = _register_dve_op(
    "TRIDIAG_PSI",
    _Spec(
        body=(_S0 * (_S0 + _TWO)) * (_One - _c_psi * (_One - _c_psi)),
        reference=_ref_psi,
    ),
)
OP_RCHI = _register_dve_op(
    "TRIDIAG_RCHI", _Spec(body=_series_chi(_sq0 * _S0) * _S1, reference=_ref_rchi)
)


def _ref_gstep1(in0, in1, c0, c1, c2):
    t = (in0.astype(_np.float32) * (1 + in1.astype(_np.float32))).astype(_np.float32)
    t2 = t * t
    return (1 + t) * (1 + t2)


_t_g1 = _S0 * (_One + _S1)
_t2_g1 = _t_g1 * _t_g1
OP_GSTEP1 = _register_dve_op(
    "TRIDIAG_GSTEP1",
    _Spec(body=(_One + _t_g1) * (_One + _t2_g1), reference=_ref_gstep1),
)


def emit_core(
    tc, alpha_in, f_in, x_out, D=D, K=K_SWEEPS, HF=HF, HB=HB, final_acc=True
):
    """Emit one core's program.

    alpha_in: dram handle [P*D + HF + HB + 2]  (alpha padded: lane p uses
              [p*D, p*D + T + 2), covering global rows g-1, g, g+1 for its
              window rows g in [p*D - HF, p*D + D + HB))
    f_in:     dram handle [P*D + HF + HB]
    x_out:    dram handle [P*D]
    """
    nc = tc.nc
    T = HF + D + HB
    TA = T + 2
    with contextlib.ExitStack() as ctx:
        pool = ctx.enter_context(tc.tile_pool(name="w", bufs=1))
        t_alpha = pool.tile([P, TA], F32, tag="alpha")  # later reused: q
        t_f = pool.tile([P, TA], F32, tag="f")          # later: B, in place
        t_sq = pool.tile([P, TA], F32, tag="sq")        # later: A, in place
        t_um = pool.tile([P, TA], F32, tag="um")        # (alpha+1)^2 - 1 via ACT
        t_ncp = pool.tile([P, TA], F32, tag="ncp")      # negcp (GPSIMD)
        t_cube = pool.tile([P, TA], F32, tag="cube")    # later: x
        t_su = pool.tile([P, TA], F32, tag="su")        # later: dp
        t_m = pool.tile([P, TA], F32, tag="m")
        t_r = pool.tile([P, TA], F32, tag="r")

        nc.sync.dma_start(t_alpha[:], bass.AP(alpha_in, 0, [[D, P], [1, TA]]))
        nc.sync.dma_start(t_f[:, 0:T], bass.AP(f_in, 0, [[D, P], [1, T]]))

        # ACT: sq = alpha^2 ; um = (alpha+1)^2 - 1 = alpha^2 + 2 alpha
        nc.scalar.activation(t_sq[:], t_alpha[:], ACTF.Square)
        nc.scalar.activation(t_um[:], t_alpha[:], ACTF.Square, bias=1.0)
        nc.scalar.activation(t_um[:], t_um[:], ACTF.Identity, bias=-1.0)
        # cube[k] = alpha[k]^3
        nc.vector.tensor_tensor(t_cube[:], t_sq[:], t_alpha[:], ALU.mult)
        # su[t] = s_t * u_{t-1} = sq[t] * um[t+1]   (row t: g = lane_base - HF + t)
        nc.vector.tensor_tensor(
            t_su[:, 0:T], t_sq[:, 0:T], t_um[:, 1 : T + 1], ALU.mult
        )
        # m^0 = b = 1 + cube[t+1]   (tensor_scalar: 2x perf mode)
        nc.vector.tensor_scalar(t_m[:, 0:T], t_cube[:, 1 : T + 1], 1.0, None, ALU.add)
        # fixed point:  m[t] = b[t] - su[t] * r[t-1],  r = 1/m
        q = t_alpha  # alpha dead after cube
        for k in range(K):
            nc.vector.reciprocal_approx_fast(out=t_r[:, 0:T], in_=t_m[:, 0:T])
            nc.vector.tensor_tensor(
                q[:, 0 : T - 1], t_su[:, 1:T], t_r[:, 0 : T - 1], ALU.mult
            )
            nc.vector.scalar_tensor_tensor(
                t_m[:, 1:T], t_cube[:, 2 : T + 1], 1.0, q[:, 0 : T - 1],
                ALU.add, ALU.subtract,
            )
        if final_acc:
            nc.vector.reciprocal_approx_accurate(
                out=t_r[:, 0:T], in_=t_m[:, 0:T], scratch=q[:, 0:T]
            )
        else:
            nc.vector.reciprocal_approx_fast(out=t_r[:, 0:T], in_=t_m[:, 0:T])

        # rn = -r  (tensor_scalar: 2x perf mode)
        rn = t_m  # m dead after final reciprocal
        nc.vector.tensor_scalar(rn[:, 0:T], t_r[:, 0:T], -1.0, None, ALU.mult)
        # negcp = -u*r = um[t+2]*rn   (GPSIMD, overlaps the dp scan)
        nc.gpsimd.tensor_tensor(
            t_ncp[:, 0:T], t_um[:, 2 : T + 2], rn[:, 0:T], ALU.mult
        )
        # A = -s*r = sq*rn ; B = f*r
        nc.vector.tensor_tensor(t_sq[:, 0:T], t_sq[:, 0:T], rn[:, 0:T], ALU.mult)
        nc.vector.tensor_tensor(t_f[:, 0:T], t_f[:, 0:T], t_r[:, 0:T], ALU.mult)
        # dp scan: dp[t] = A[t]*dp[t-1] + B[t]
        nc.vector.tensor_tensor_scan(
            t_su[:, 0:T], t_sq[:, 0:T], t_f[:, 0:T], 0.0, ALU.mult, ALU.add
        )
        # backward scan (reversed): x[t] = negcp[t]*x[t+1] + dp[t]
        nc.vector.tensor_tensor_scan(
            t_cube[:, 0:T][:, ::-1],
            t_ncp[:, 0:T][:, ::-1],
            t_su[:, 0:T][:, ::-1],
            0.0,
            ALU.mult,
            ALU.add,
        )
        nc.sync.dma_start(
            bass.AP(x_out, 0, [[D, P], [1, D]]), t_cube[:, HF : HF + D]
        )


def emit_core_g(tc, alpha_in, f_in, x_out, D=D, K=K_G, HF=HF, HB=HB):
    """g-form pipeline: custom-DVE series ops, no reciprocal instructions.

    rho_t = su_t/(b_t b_{t-1}) = (sq[t] X[t]) * (umm[t+1] X[t+1]),
    g = 1/mu via fixed point  g <- (1+t)(1+t^2)(1+t^4), t = rho*g_prev,
    r_t = 1/m_t = X[t+1] * g_t.
    """
    nc = tc.nc
    T = HF + D + HB
    TA = T + 2
    with contextlib.ExitStack() as ctx:
        pool = ctx.enter_context(tc.tile_pool(name="w", bufs=1))
        t_alpha = pool.tile([P, TA], F32, tag="alpha")
        t_f = pool.tile([P, TA], F32, tag="f")      # -> B in place
        t_sq = pool.tile([P, TA], F32, tag="sq")    # -> A in place
        t_u1 = pool.tile([P, TA], F32, tag="u1")    # (alpha+1)^2; -> negcp
        t_pp = pool.tile([P, TA], F32, tag="pp")    # -> rho in place -> x (scan2)
        t_qq = pool.tile([P, TA], F32, tag="qq")    # -> dp (scan1)
        t_g = pool.tile([P, TA], F32, tag="g")
        t_r = pool.tile([P, TA], F32, tag="r")

        # Head is column-chunked so ACT/DVE start on the first part of alpha
        # while the rest is still in flight.
        ncol = NCOL_HEAD
        if GEO_HEAD:
            w0 = TA // (2 ** ncol - 1)
            edges = [0]
            for j in range(ncol - 1):
                edges.append(edges[-1] + w0 * (2 ** j))
            edges.append(TA)
        else:
            edges = [0] + [TA * (j + 1) // ncol for j in range(ncol)]
        for j in range(ncol):
            lo, hi = edges[j], edges[j + 1]
            dma_eng = nc.sync if j % 2 == 0 else nc.scalar
            dma_eng.dma_start(
                t_alpha[:, lo:hi], bass.AP(alpha_in, lo, [[D, P], [1, hi - lo]])
            )
            # ACT: sq = alpha^2 ; u1 = (alpha+1)^2 = alpha^2 + 2 alpha + 1
            nc.scalar.activation(t_sq[:, lo:hi], t_alpha[:, lo:hi], ACTF.Square)
            nc.scalar.activation(
                t_u1[:, lo:hi], t_alpha[:, lo:hi], ACTF.Square, bias=1.0
            )
            if HEAD_V2:
                # PP = phi(alpha) = alpha^2 chi(alpha); QQ = psi(alpha)
                nc.vector._custom_dve(
                    OP_PHI, out=t_pp[:, lo:hi], in0=t_alpha[:, lo:hi]
                )
                nc.vector._custom_dve(
                    OP_PSI, out=t_qq[:, lo:hi], in0=t_alpha[:, lo:hi]
                )
            else:
                nc.vector._custom_dve(
                    OP_CHI, out=t_r[:, lo:hi], in0=t_alpha[:, lo:hi]
                )
                nc.vector.tensor_tensor(
                    t_pp[:, lo:hi], t_sq[:, lo:hi], t_r[:, lo:hi], ALU.mult
                )
                nc.vector._custom_dve(
                    OP_SUBONE_MUL, out=t_qq[:, lo:hi], in0=t_u1[:, lo:hi],
                    in1=t_r[:, lo:hi],
                )
        # rho_t = PP[t]*QQ[t+1]
        rho = t_pp[:, 0:T]
        nc.vector.tensor_tensor(rho, t_pp[:, 0:T], t_qq[:, 1 : T + 1], ALU.mult)
        # g fixed point.  Init g0 = 1 + rho (tensor_scalar: 2x perf mode);
        # each GS_STEP then applies the exact truncated map in place
        # (the in-place shifted read sees the previous sweep's values:
        # position t-1 is read two cycles before its new value lands).
        if GS_TS_INIT == "gstep1":
            # g[0] = 1 + rho[0];  g[1:] = telescope3(rho*(1+rho@-1))
            # (two map applications in one 6-stage op)
            nc.vector.tensor_scalar(
                t_g[:, 0:1], t_pp[:, 0:1], 1.0, None, ALU.add
            )
            nc.vector._custom_dve(
                OP_GSTEP1, out=t_g[:, 1:T], in0=t_pp[:, 1:T],
                in1=t_pp[:, 0 : T - 1],
            )
            nsteps = K - 2
        elif GS_TS_INIT:
            nc.vector.tensor_scalar(t_g[:, 0:T], rho, 1.0, None, ALU.add)
            nsteps = K - 1
        else:
            nc.vector._custom_dve(OP_GS_INIT, out=t_g[:, 0:T], in0=rho)
            nsteps = K - 1
        for _ in range(nsteps):
            nc.vector._custom_dve(
                OP_GS_STEP, out=t_g[:, 1:T], in0=t_pp[:, 1:T],
                in1=t_g[:, 0 : T - 1],
            )
        # f arrives late: its only consumer (B) is ~60us into the pipeline,
        # so don't let it compete with the alpha load at kernel start.
        nc.sync.dma_start(t_f[:, 0:T], bass.AP(f_in, 0, [[D, P], [1, T]]))
        # r = chi(alpha[t+1])*g
        if HEAD_V2:
            nc.vector._custom_dve(
                OP_RCHI, out=t_r[:, 0:T], in0=t_alpha[:, 1 : T + 1],
                in1=t_g[:, 0:T],
            )
        else:
            # t_r holds X = chi(alpha); write r over it (write trails read)
            nc.vector.tensor_tensor(
                t_r[:, 0:T], t_r[:, 1 : T + 1], t_g[:, 0:T], ALU.mult
            )
        # A = -sq*r (in place) ; B = f*r (in place) ; negcp = (1-u1[t+2])*r
        nc.vector._custom_dve(
            OP_NEGMUL, out=t_sq[:, 0:T], in0=t_sq[:, 0:T], in1=t_r[:, 0:T]
        )
        nc.vector.tensor_tensor(t_f[:, 0:T], t_f[:, 0:T], t_r[:, 0:T], ALU.mult)
        nc.vector._custom_dve(
            OP_ONESUB_MUL, out=t_u1[:, 0:T], in0=t_u1[:, 2 : T + 2], in1=t_r[:, 0:T]
        )
        # dp scan then reversed back-substitution scan.  The reversed scan is
        # split into column chunks (high chunk first) chained through
        # `initial`, so each chunk's output DMA overlaps the next chunk's scan.
        nc.vector.tensor_tensor_scan(
            t_qq[:, 0:T], t_sq[:, 0:T], t_f[:, 0:T], 0.0, ALU.mult, ALU.add
        )
        nchunk = SCAN2_CHUNKS
        cuts = [0] + [HF + (D * (j + 1)) // nchunk for j in range(nchunk - 1)] + [T]
        for ci in range(len(cuts) - 2, -1, -1):
            lo, hi = cuts[ci], cuts[ci + 1]
            init = 0.0 if hi == T else t_pp[:, hi : hi + 1]
            nc.vector.tensor_tensor_scan(
                t_pp[:, lo:hi][:, ::-1],
                t_u1[:, lo:hi][:, ::-1],
                t_qq[:, lo:hi][:, ::-1],
                init,
                ALU.mult,
                ALU.add,
            )
            slo, shi = max(lo, HF), min(hi, HF + D)
            nc.sync.dma_start(
                bass.AP(x_out, slo - HF, [[D, P], [1, shi - slo]]),
                t_pp[:, slo:shi],
            )


ALGO = "g"  # "g" (custom-op pipeline) or "v1" (stock-op pipeline)
HEAD_V2 = True  # PHI/PSI/RCHI fused head vs CHI+tt head
NCOL_HEAD = 4
SCAN2_CHUNKS = 4
GEO_HEAD = False
GS_TS_INIT = "gstep1"  # "gstep1" | True (ts 1+rho) | False (GS_INIT custom)


def build_nc(D=D, K=K_SWEEPS, HF=HF, HB=HB, ncores=NCORES, final_acc=True):
    C = P * D
    nc = bacc.Bacc(
        "TRN2", target_bir_lowering=False, debug=False, num_devices=ncores
    )
    alpha_in = nc.dram_tensor("alpha_in", [C + HF + HB + 2], F32, kind="ExternalInput")
    f_in = nc.dram_tensor("f_in", [C + HF + HB], F32, kind="ExternalInput")
    x_out = nc.dram_tensor("x_out", [C], F32, kind="ExternalOutput")
    with tile.TileContext(nc) as tc:
        if ALGO == "g":
            emit_core_g(tc, alpha_in, f_in, x_out, D=D, K=K_G, HF=HF, HB=HB)
        else:
            emit_core(tc, alpha_in, f_in, x_out, D=D, K=K, HF=HF, HB=HB,
                      final_acc=final_acc)
    nc.compile()
    return nc


def shard_inputs(alpha, f, D=D, HF=HF, HB=HB, ncores=NCORES):
    C = P * D
    n = ncores * C
    alpha_pad = np.zeros(n + HF + HB + 2, dtype=np.float32)
    alpha_pad[HF + 1 : HF + 1 + n] = alpha
    f_pad = np.zeros(n + HF + HB, dtype=np.float32)
    f_pad[HF : HF + n] = f
    in_maps = []
    for c in range(ncores):
        in_maps.append(
            {
                "alpha_in": np.ascontiguousarray(alpha_pad[c * C : c * C + C + HF + HB + 2]),
                "f_in": np.ascontiguousarray(f_pad[c * C : c * C + C + HF + HB]),
            }
        )
    return in_maps


_NC_CACHE = {}


def kernel(alpha: np.ndarray, f: np.ndarray, trace: bool = False, **run_kwargs):
    from concourse import bass_utils

    alpha = np.asarray(alpha, dtype=np.float32)
    f = np.asarray(f, dtype=np.float32)
    assert alpha.shape == (N,) and f.shape == (N,)
    key = (D, K_SWEEPS, HF, HB, FINAL_ACC, ALGO, K_G, HEAD_V2, NCOL_HEAD, SCAN2_CHUNKS, GS_TS_INIT, GEO_HEAD)
    if key not in _NC_CACHE:
        _NC_CACHE[key] = build_nc(K=K_SWEEPS, final_acc=FINAL_ACC)
    nc = _NC_CACHE[key]
    in_maps = shard_inputs(alpha, f)
    res = bass_utils.run_bass_kernel_spmd(
        nc, in_maps, core_ids=list(range(NCORES)), trace=trace, **run_kwargs
    )
    out = np.concatenate([res.results[c]["x_out"] for c in range(NCORES)])
    if trace:
        kernel.last_results = res
    return out


# revision 50
# speedup vs baseline: 1.0354x; 1.0144x over previous
"""Trainium2 Bass kernel: tridiagonal solve A(alpha) x = f, N = 4M, f32.

A is strictly diagonally dominant (b = 1+alpha^3 >= 1, sub = alpha^2 <= 0.09,
super = alpha^2 + 2 alpha <= 0.69 for alpha in [0, 0.3)).  All Thomas-algorithm
recurrences therefore forget their initial conditions at a geometric rate
(forward: |a/m| <= 0.097 per step, backward: |cp| <= 0.74 per step), so the
global sequential solve can be replaced by fully independent overlapping
windows: each of 8 cores x 128 lanes owns a contiguous 4096-element chunk and
computes it exactly (to fp32) using a 16-element forward warmup halo and a
64-element backward warmup halo.  No reduced system, no collectives.

The affine recurrences (forward-eliminated rhs dp, and back-substitution)
run on the DVE tensor_tensor_scan instruction (state = d0*state + d1 per
partition along the free dim; the back-substitution scan uses negative-stride
APs to run in reverse).  The nonlinear pivot recurrence, normalized as
g_t = 1/(1 - rho_t g_{t-1}) with rho_t = su_t/(b_t b_{t-1}) in [0, 0.075],
is solved by Jacobi sweeps where one custom 8-stage DVE op applies the whole
map exactly: the degree-7 truncation (1+t)(1+t^2)(1+t^4) of 1/(1-t) is
fp32-exact for t <= 0.075.  1/b = chi(alpha) is likewise an fp32-exact cubic
series.  No reciprocal/divide instructions anywhere.

Measured on trn2 (8 cores): ~87.5 us NEFF exec, absmax/scale error ~8.5e-7
(the fp32 reference itself is ~1.6e-7 from float64).
"""

import contextlib

import numpy as np

import concourse.bacc as bacc
import concourse.bass as bass
import concourse.mybir as mybir
import concourse.tile as tile

N = 4_194_304
NCORES = 8
P = 128
D = N // (NCORES * P)  # 4096 elements per lane
HF = 16   # forward warmup halo
HB = 64   # backward warmup halo
K_SWEEPS = 3  # fixed-point sweeps for the pivot sequence m (v1 algo)
FINAL_ACC = True  # accurate (2-op) final reciprocal vs fast (1-op)  (v1 algo)
K_G = 4   # total g-updates (init + steps) in the g-form algo
F32 = mybir.dt.float32
ALU = mybir.AluOpType
ACTF = mybir.ActivationFunctionType

# ---------------------------------------------------------------------------
# Custom DVE ops for the g-form pipeline.
#
# The normalized pivot recurrence is mu_t = 1 - rho_t / mu_{t-1} with
# rho_t = su_t / (b_t b_{t-1}) in [0, 0.075].  Iterating directly on
# g := 1/mu:  g_t = 1/(1 - rho_t g_{t-1}).  Since t = rho*g <= 0.075, the
# degree-7 truncation (1+t)(1+t^2)(1+t^4) of 1/(1-t) is exact to fp32
# (t^8 < 1e-9), so one 8-stage custom DVE op implements a whole sweep with
# no reciprocal anywhere.  chi(x) = 1/(1+x^3) by the same argument
# (x^3 <= 0.027, quartic-term error < 6e-7).
# ---------------------------------------------------------------------------
import numpy as _np
from concourse import dve_ops as _dvo
from concourse.dve_spec import Spec as _Spec, Src0 as _S0, Src1 as _S1, One as _One
from concourse.dve_spec import lower as _dve_lower, _has_src1
from concourse.dve_table_gen import dve_ver_for as _dve_ver_for
from concourse.dve_uop import DveOpSpec as _DveOpSpec


def _register_dve_op(name, spec, subdim=False):
    existing = {op.name: op for op in _dvo.OPS}
    if name in existing:
        return existing[name]
    row = max(_dvo._SUB_OPCODE_FOR_NAME.values()) + 1
    assert row < 0x20
    shas = {}
    for ver in ("v3", "v4"):
        compiled = _DveOpSpec(
            name=name, opcode=row, uops=_dve_lower(spec, ver=ver),
            rd1_en=_has_src1(spec),
        )
        shas[ver] = compiled.sha(ver)
        _dvo._COMPILE_CACHE[(name, ver)] = compiled
    op = _dvo.DveOp(name, spec, subdim=subdim, uops_sha=shas)
    _dvo.OPS.append(op)
    _dvo._SUB_OPCODE_FOR_NAME[name] = row
    _dvo.CUSTOM_DVE_SPECS[name] = spec
    return op


def _series_chi(c):
    # 1 - c + c^2 - c^3 = 1 - c*(1 - c*(1 - c))
    return _One - c * (_One - c * (_One - c))


def _ref_chi(in0, in1, c0, c1, c2):
    x = in0.astype(_np.float32)
    c = x * x * x
    one = _np.float32(1.0)
    return one - c * (one - c * (one - c))


def _telescope(t):
    # (1+t)(1+t^2)(1+t^4) = sum_{j=0..7} t^j  ~= 1/(1-t) for |t| << 1
    t2 = t * t
    t4 = t2 * t2
    return ((_One + t) * (_One + t2)) * (_One + t4)


def _ref_gs_init(in0, in1, c0, c1, c2):
    t = in0.astype(_np.float32)
    t2 = t * t
    t4 = t2 * t2
    return ((1 + t) * (1 + t2)) * (1 + t4)


def _ref_gs_step(in0, in1, c0, c1, c2):
    t = (in0.astype(_np.float32) * in1.astype(_np.float32)).astype(_np.float32)
    t2 = t * t
    t4 = t2 * t2
    return ((1 + t) * (1 + t2)) * (1 + t4)


def _ref_negmul(in0, in1, c0, c1, c2):
    return -(in0.astype(_np.float32) * in1.astype(_np.float32))


_sq0 = _S0 * _S0
OP_CHI = _register_dve_op(
    "TRIDIAG_CHI", _Spec(body=_series_chi(_sq0 * _S0), reference=_ref_chi)
)
OP_GS_INIT = _register_dve_op(
    "TRIDIAG_GS_INIT", _Spec(body=_telescope(_S0), reference=_ref_gs_init)
)
OP_GS_STEP = _register_dve_op(
    "TRIDIAG_GS_STEP", _Spec(body=_telescope(_S0 * _S1), reference=_ref_gs_step)
)
OP_NEGMUL = _register_dve_op(
    "TRIDIAG_NEGMUL", _Spec(body=-(_S0 * _S1), reference=_ref_negmul)
)


def _ref_subone_mul(in0, in1, c0, c1, c2):
    return (in0.astype(_np.float32) - _np.float32(1.0)) * in1.astype(_np.float32)


def _ref_onesub_mul(in0, in1, c0, c1, c2):
    return (_np.float32(1.0) - in0.astype(_np.float32)) * in1.astype(_np.float32)


OP_SUBONE_MUL = _register_dve_op(
    "TRIDIAG_SUBONE_MUL", _Spec(body=(_S0 - _One) * _S1, reference=_ref_subone_mul)
)
OP_ONESUB_MUL = _register_dve_op(
    "TRIDIAG_ONESUB_MUL", _Spec(body=(_One - _S0) * _S1, reference=_ref_onesub_mul)
)


def _np_chi(x, terms=3):
    c = (x * x * x).astype(_np.float32)
    one = _np.float32(1.0)
    if terms == 3:
        return one - c * (one - c * (one - c))
    return one - c * (one - c)


def _ref_phi(in0, in1, c0, c1, c2):
    x = in0.astype(_np.float32)
    return (x * x) * _np_chi(x)


def _ref_psi(in0, in1, c0, c1, c2):
    x = in0.astype(_np.float32)
    return (x * (x + _np.float32(2.0))) * _np_chi(x, terms=2)


def _ref_rchi(in0, in1, c0, c1, c2):
    x = in0.astype(_np.float32)
    return _np_chi(x) * in1.astype(_np.float32)


_TWO = _One + _One
OP_PHI = _register_dve_op(
    "TRIDIAG_PHI", _Spec(body=_series_chi(_sq0 * _S0) * _sq0, reference=_ref_phi)
)
_c_psi = _sq0 * _S0
OP_PSI = _register_dve_op(
    "TRIDIAG_PSI",
    _Spec(
        body=(_S0 * (_S0 + _TWO)) * (_One - _c_psi * (_One - _c_psi)),
        reference=_ref_psi,
    ),
)
OP_RCHI = _register_dve_op(
    "TRIDIAG_RCHI", _Spec(body=_series_chi(_sq0 * _S0) * _S1, reference=_ref_rchi)
)


def _ref_gstep1(in0, in1, c0, c1, c2):
    t = (in0.astype(_np.float32) * (1 + in1.astype(_np.float32))).astype(_np.float32)
    t2 = t * t
    return (1 + t) * (1 + t2)


_t_g1 = _S0 * (_One + _S1)
_t2_g1 = _t_g1 * _t_g1
OP_GSTEP1 = _register_dve_op(
    "TRIDIAG_GSTEP1",
    _Spec(body=(_One + _t_g1) * (_One + _t2_g1), reference=_ref_gstep1),
)


def emit_core(
    tc, alpha_in, f_in, x_out, D=D, K=K_SWEEPS, HF=HF, HB=HB, final_acc=True
):
    """Emit one core's program.

    alpha_in: dram handle [P*D + HF + HB + 2]  (alpha padded: lane p uses
              [p*D, p*D + T + 2), covering global rows g-1, g, g+1 for its
              window rows g in [p*D - HF, p*D + D + HB))
    f_in:     dram handle [P*D + HF + HB]
    x_out:    dram handle [P*D]
    """
    nc = tc.nc
    T = HF + D + HB
    TA = T + 2
    with contextlib.ExitStack() as ctx:
        pool = ctx.enter_context(tc.tile_pool(name="w", bufs=1))
        t_alpha = pool.tile([P, TA], F32, tag="alpha")  # later reused: q
        t_f = pool.tile([P, TA], F32, tag="f")          # later: B, in place
        t_sq = pool.tile([P, TA], F32, tag="sq")        # later: A, in place
        t_um = pool.tile([P, TA], F32, tag="um")        # (alpha+1)^2 - 1 via ACT
        t_ncp = pool.tile([P, TA], F32, tag="ncp")      # negcp (GPSIMD)
        t_cube = pool.tile([P, TA], F32, tag="cube")    # later: x
        t_su = pool.tile([P, TA], F32, tag="su")        # later: dp
        t_m = pool.tile([P, TA], F32, tag="m")
        t_r = pool.tile([P, TA], F32, tag="r")

        nc.sync.dma_start(t_alpha[:], bass.AP(alpha_in, 0, [[D, P], [1, TA]]))
        nc.sync.dma_start(t_f[:, 0:T], bass.AP(f_in, 0, [[D, P], [1, T]]))

        # ACT: sq = alpha^2 ; um = (alpha+1)^2 - 1 = alpha^2 + 2 alpha
        nc.scalar.activation(t_sq[:], t_alpha[:], ACTF.Square)
        nc.scalar.activation(t_um[:], t_alpha[:], ACTF.Square, bias=1.0)
        nc.scalar.activation(t_um[:], t_um[:], ACTF.Identity, bias=-1.0)
        # cube[k] = alpha[k]^3
        nc.vector.tensor_tensor(t_cube[:], t_sq[:], t_alpha[:], ALU.mult)
        # su[t] = s_t * u_{t-1} = sq[t] * um[t+1]   (row t: g = lane_base - HF + t)
        nc.vector.tensor_tensor(
            t_su[:, 0:T], t_sq[:, 0:T], t_um[:, 1 : T + 1], ALU.mult
        )
        # m^0 = b = 1 + cube[t+1]   (tensor_scalar: 2x perf mode)
        nc.vector.tensor_scalar(t_m[:, 0:T], t_cube[:, 1 : T + 1], 1.0, None, ALU.add)
        # fixed point:  m[t] = b[t] - su[t] * r[t-1],  r = 1/m
        q = t_alpha  # alpha dead after cube
        for k in range(K):
            nc.vector.reciprocal_approx_fast(out=t_r[:, 0:T], in_=t_m[:, 0:T])
            nc.vector.tensor_tensor(
                q[:, 0 : T - 1], t_su[:, 1:T], t_r[:, 0 : T - 1], ALU.mult
            )
            nc.vector.scalar_tensor_tensor(
                t_m[:, 1:T], t_cube[:, 2 : T + 1], 1.0, q[:, 0 : T - 1],
                ALU.add, ALU.subtract,
            )
        if final_acc:
            nc.vector.reciprocal_approx_accurate(
                out=t_r[:, 0:T], in_=t_m[:, 0:T], scratch=q[:, 0:T]
            )
        else:
            nc.vector.reciprocal_approx_fast(out=t_r[:, 0:T], in_=t_m[:, 0:T])

        # rn = -r  (tensor_scalar: 2x perf mode)
        rn = t_m  # m dead after final reciprocal
        nc.vector.tensor_scalar(rn[:, 0:T], t_r[:, 0:T], -1.0, None, ALU.mult)
        # negcp = -u*r = um[t+2]*rn   (GPSIMD, overlaps the dp scan)
        nc.gpsimd.tensor_tensor(
            t_ncp[:, 0:T], t_um[:, 2 : T + 2], rn[:, 0:T], ALU.mult
        )
        # A = -s*r = sq*rn ; B = f*r
        nc.vector.tensor_tensor(t_sq[:, 0:T], t_sq[:, 0:T], rn[:, 0:T], ALU.mult)
        nc.vector.tensor_tensor(t_f[:, 0:T], t_f[:, 0:T], t_r[:, 0:T], ALU.mult)
        # dp scan: dp[t] = A[t]*dp[t-1] + B[t]
        nc.vector.tensor_tensor_scan(
            t_su[:, 0:T], t_sq[:, 0:T], t_f[:, 0:T], 0.0, ALU.mult, ALU.add
        )
        # backward scan (reversed): x[t] = negcp[t]*x[t+1] + dp[t]
        nc.vector.tensor_tensor_scan(
            t_cube[:, 0:T][:, ::-1],
            t_ncp[:, 0:T][:, ::-1],
            t_su[:, 0:T][:, ::-1],
            0.0,
            ALU.mult,
            ALU.add,
        )
        nc.sync.dma_start(
            bass.AP(x_out, 0, [[D, P], [1, D]]), t_cube[:, HF : HF + D]
        )


def emit_core_g(tc, alpha_in, f_in, x_out, D=D, K=K_G, HF=HF, HB=HB):
    """g-form pipeline: custom-DVE series ops, no reciprocal instructions.

    rho_t = su_t/(b_t b_{t-1}) = (sq[t] X[t]) * (umm[t+1] X[t+1]),
    g = 1/mu via fixed point  g <- (1+t)(1+t^2)(1+t^4), t = rho*g_prev,
    r_t = 1/m_t = X[t+1] * g_t.
    """
    nc = tc.nc
    T = HF + D + HB
    TA = T + 2
    with contextlib.ExitStack() as ctx:
        pool = ctx.enter_context(tc.tile_pool(name="w", bufs=1))
        t_alpha = pool.tile([P, TA], F32, tag="alpha")
        t_f = pool.tile([P, TA], F32, tag="f")      # -> B in place
        t_sq = pool.tile([P, TA], F32, tag="sq")    # -> A in place
        t_u1 = pool.tile([P, TA], F32, tag="u1")    # (alpha+1)^2; -> negcp
        t_pp = pool.tile([P, TA], F32, tag="pp")    # -> rho in place -> x (scan2)
        t_qq = pool.tile([P, TA], F32, tag="qq")    # -> dp (scan1)
        t_g = pool.tile([P, TA], F32, tag="g")
        t_r = pool.tile([P, TA], F32, tag="r")

        # Head is column-chunked so ACT/DVE start on the first part of alpha
        # while the rest is still in flight.
        ncol = NCOL_HEAD
        if GEO_HEAD:
            w0 = TA // (2 ** ncol - 1)
            edges = [0]
            for j in range(ncol - 1):
                edges.append(edges[-1] + w0 * (2 ** j))
            edges.append(TA)
        else:
            edges = [0] + [TA * (j + 1) // ncol for j in range(ncol)]
        for j in range(ncol):
            lo, hi = edges[j], edges[j + 1]
            dma_eng = nc.sync if j % 2 == 0 else nc.scalar
            dma_eng.dma_start(
                t_alpha[:, lo:hi], bass.AP(alpha_in, lo, [[D, P], [1, hi - lo]])
            )
            # ACT: sq = alpha^2 ; u1 = (alpha+1)^2 = alpha^2 + 2 alpha + 1
            nc.scalar.activation(t_sq[:, lo:hi], t_alpha[:, lo:hi], ACTF.Square)
            nc.scalar.activation(
                t_u1[:, lo:hi], t_alpha[:, lo:hi], ACTF.Square, bias=1.0
            )
            if HEAD_V2:
                # PP = phi(alpha) = alpha^2 chi(alpha); QQ = psi(alpha)
                nc.vector._custom_dve(
                    OP_PHI, out=t_pp[:, lo:hi], in0=t_alpha[:, lo:hi]
                )
                nc.vector._custom_dve(
                    OP_PSI, out=t_qq[:, lo:hi], in0=t_alpha[:, lo:hi]
                )
            else:
                nc.vector._custom_dve(
                    OP_CHI, out=t_r[:, lo:hi], in0=t_alpha[:, lo:hi]
                )
                nc.vector.tensor_tensor(
                    t_pp[:, lo:hi], t_sq[:, lo:hi], t_r[:, lo:hi], ALU.mult
                )
                nc.vector._custom_dve(
                    OP_SUBONE_MUL, out=t_qq[:, lo:hi], in0=t_u1[:, lo:hi],
                    in1=t_r[:, lo:hi],
                )
        # rho_t = PP[t]*QQ[t+1]
        rho = t_pp[:, 0:T]
        nc.vector.tensor_tensor(rho, t_pp[:, 0:T], t_qq[:, 1 : T + 1], ALU.mult)
        # g fixed point.  Init g0 = 1 + rho (tensor_scalar: 2x perf mode);
        # each GS_STEP then applies the exact truncated map in place
        # (the in-place shifted read sees the previous sweep's values:
        # position t-1 is read two cycles before its new value lands).
        if GS_TS_INIT == "gstep1":
            # g[0] = 1 + rho[0];  g[1:] = telescope3(rho*(1+rho@-1))
            # (two map applications in one 6-stage op)
            nc.vector.tensor_scalar(
                t_g[:, 0:1], t_pp[:, 0:1], 1.0, None, ALU.add
            )
            nc.vector._custom_dve(
                OP_GSTEP1, out=t_g[:, 1:T], in0=t_pp[:, 1:T],
                in1=t_pp[:, 0 : T - 1],
            )
            nsteps = K - 2
        elif GS_TS_INIT:
            nc.vector.tensor_scalar(t_g[:, 0:T], rho, 1.0, None, ALU.add)
            nsteps = K - 1
        else:
            nc.vector._custom_dve(OP_GS_INIT, out=t_g[:, 0:T], in0=rho)
            nsteps = K - 1
        for _ in range(nsteps):
            nc.vector._custom_dve(
                OP_GS_STEP, out=t_g[:, 1:T], in0=t_pp[:, 1:T],
                in1=t_g[:, 0 : T - 1],
            )
        # f arrives late: its only consumer (B) is ~60us into the pipeline,
        # so don't let it compete with the alpha load at kernel start.
        nc.sync.dma_start(t_f[:, 0:T], bass.AP(f_in, 0, [[D, P], [1, T]]))
        # r = chi(alpha[t+1])*g
        if HEAD_V2:
            nc.vector._custom_dve(
                OP_RCHI, out=t_r[:, 0:T], in0=t_alpha[:, 1 : T + 1],
                in1=t_g[:, 0:T],
            )
        else:
            # t_r holds X = chi(alpha); write r over it (write trails read)
            nc.vector.tensor_tensor(
                t_r[:, 0:T], t_r[:, 1 : T + 1], t_g[:, 0:T], ALU.mult
            )
        # A = -sq*r (in place) ; B = f*r (in place) ; negcp = (1-u1[t+2])*r
        nc.vector._custom_dve(
            OP_NEGMUL, out=t_sq[:, 0:T], in0=t_sq[:, 0:T], in1=t_r[:, 0:T]
        )
        nc.vector.tensor_tensor(t_f[:, 0:T], t_f[:, 0:T], t_r[:, 0:T], ALU.mult)
        nc.vector._custom_dve(
            OP_ONESUB_MUL, out=t_u1[:, 0:T], in0=t_u1[:, 2 : T + 2], in1=t_r[:, 0:T]
        )
        # dp scan then reversed back-substitution scan.  The reversed scan is
        # split into column chunks (high chunk first) chained through
        # `initial`, so each chunk's output DMA overlaps the next chunk's scan.
        nc.vector.tensor_tensor_scan(
            t_qq[:, 0:T], t_sq[:, 0:T], t_f[:, 0:T], 0.0, ALU.mult, ALU.add
        )
        nchunk = SCAN2_CHUNKS
        cuts = [0] + [HF + (D * (j + 1)) // nchunk for j in range(nchunk - 1)] + [T]
        for ci in range(len(cuts) - 2, -1, -1):
            lo, hi = cuts[ci], cuts[ci + 1]
            init = 0.0 if hi == T else t_pp[:, hi : hi + 1]
            nc.vector.tensor_tensor_scan(
                t_pp[:, lo:hi][:, ::-1],
                t_u1[:, lo:hi][:, ::-1],
                t_qq[:, lo:hi][:, ::-1],
                init,
                ALU.mult,
                ALU.add,
            )
            slo, shi = max(lo, HF), min(hi, HF + D)
            nc.sync.dma_start(
                bass.AP(x_out, slo - HF, [[D, P], [1, shi - slo]]),
                t_pp[:, slo:shi],
            )


ALGO = "g"  # "g" (custom-op pipeline) or "v1" (stock-op pipeline)
HEAD_V2 = True  # PHI/PSI/RCHI fused head vs CHI+tt head
NCOL_HEAD = 3
SCAN2_CHUNKS = 4
GEO_HEAD = False
GS_TS_INIT = "gstep1"  # "gstep1" | True (ts 1+rho) | False (GS_INIT custom)


def build_nc(D=D, K=K_SWEEPS, HF=HF, HB=HB, ncores=NCORES, final_acc=True):
    C = P * D
    nc = bacc.Bacc(
        "TRN2", target_bir_lowering=False, debug=False, num_devices=ncores
    )
    alpha_in = nc.dram_tensor("alpha_in", [C + HF + HB + 2], F32, kind="ExternalInput")
    f_in = nc.dram_tensor("f_in", [C + HF + HB], F32, kind="ExternalInput")
    x_out = nc.dram_tensor("x_out", [C], F32, kind="ExternalOutput")
    with tile.TileContext(nc) as tc:
        if ALGO == "g":
            emit_core_g(tc, alpha_in, f_in, x_out, D=D, K=K_G, HF=HF, HB=HB)
        else:
            emit_core(tc, alpha_in, f_in, x_out, D=D, K=K, HF=HF, HB=HB,
                      final_acc=final_acc)
    nc.compile()
    return nc


def shard_inputs(alpha, f, D=D, HF=HF, HB=HB, ncores=NCORES):
    C = P * D
    n = ncores * C
    alpha_pad = np.zeros(n + HF + HB + 2, dtype=np.float32)
    alpha_pad[HF + 1 : HF + 1 + n] = alpha
    f_pad = np.zeros(n + HF + HB, dtype=np.float32)
    f_pad[HF : HF + n] = f
    in_maps = []
    for c in range(ncores):
        in_maps.append(
            {
                "alpha_in": np.ascontiguousarray(alpha_pad[c * C : c * C + C + HF + HB + 2]),
                "f_in": np.ascontiguousarray(f_pad[c * C : c * C + C + HF + HB]),
            }
        )
    return in_maps


_NC_CACHE = {}


def kernel(alpha: np.ndarray, f: np.ndarray, trace: bool = False, **run_kwargs):
    from concourse import bass_utils

    alpha = np.asarray(alpha, dtype=np.float32)
    f = np.asarray(f, dtype=np.float32)
    assert alpha.shape == (N,) and f.shape == (N,)
    key = (D, K_SWEEPS, HF, HB, FINAL_ACC, ALGO, K_G, HEAD_V2, NCOL_HEAD, SCAN2_CHUNKS, GS_TS_INIT, GEO_HEAD)
    if key not in _NC_CACHE:
        _NC_CACHE[key] = build_nc(K=K_SWEEPS, final_acc=FINAL_ACC)
    nc = _NC_CACHE[key]
    in_maps = shard_inputs(alpha, f)
    res = bass_utils.run_bass_kernel_spmd(
        nc, in_maps, core_ids=list(range(NCORES)), trace=trace, **run_kwargs
    )
    out = np.concatenate([res.results[c]["x_out"] for c in range(NCORES)])
    if trace:
        kernel.last_results = res
    return out


# revision 51
# speedup vs baseline: 1.0462x; 1.0104x over previous
"""Trainium2 Bass kernel: tridiagonal solve A(alpha) x = f, N = 4M, f32.

A is strictly diagonally dominant (b = 1+alpha^3 >= 1, sub = alpha^2 <= 0.09,
super = alpha^2 + 2 alpha <= 0.69 for alpha in [0, 0.3)).  All Thomas-algorithm
recurrences therefore forget their initial conditions at a geometric rate
(forward: |a/m| <= 0.097 per step, backward: |cp| <= 0.74 per step), so the
global sequential solve can be replaced by fully independent overlapping
windows: each of 8 cores x 128 lanes owns a contiguous 4096-element chunk and
computes it exactly (to fp32) using a 16-element forward warmup halo and a
64-element backward warmup halo.  No reduced system, no collectives.

The affine recurrences (forward-eliminated rhs dp, and back-substitution)
run on the DVE tensor_tensor_scan instruction (state = d0*state + d1 per
partition along the free dim; the back-substitution scan uses negative-stride
APs to run in reverse).  The nonlinear pivot recurrence, normalized as
g_t = 1/(1 - rho_t g_{t-1}) with rho_t = su_t/(b_t b_{t-1}) in [0, 0.075],
is solved by Jacobi sweeps where one custom 8-stage DVE op applies the whole
map exactly: the degree-7 truncation (1+t)(1+t^2)(1+t^4) of 1/(1-t) is
fp32-exact for t <= 0.075.  1/b = chi(alpha) is likewise an fp32-exact cubic
series.  No reciprocal/divide instructions anywhere.

Measured on trn2 (8 cores): ~82 us NEFF exec, absmax/scale error ~8.5e-7
(the fp32 reference itself is ~1.6e-7 from float64).
"""

import contextlib

import numpy as np

import concourse.bacc as bacc
import concourse.bass as bass
import concourse.mybir as mybir
import concourse.tile as tile

N = 4_194_304
NCORES = 8
P = 128
D = N // (NCORES * P)  # 4096 elements per lane
HF = 16   # forward warmup halo
HB = 64   # backward warmup halo
K_SWEEPS = 3  # fixed-point sweeps for the pivot sequence m (v1 algo)
FINAL_ACC = True  # accurate (2-op) final reciprocal vs fast (1-op)  (v1 algo)
K_G = 4   # total g-updates (init + steps) in the g-form algo
F32 = mybir.dt.float32
ALU = mybir.AluOpType
ACTF = mybir.ActivationFunctionType

# ---------------------------------------------------------------------------
# Custom DVE ops for the g-form pipeline.
#
# The normalized pivot recurrence is mu_t = 1 - rho_t / mu_{t-1} with
# rho_t = su_t / (b_t b_{t-1}) in [0, 0.075].  Iterating directly on
# g := 1/mu:  g_t = 1/(1 - rho_t g_{t-1}).  Since t = rho*g <= 0.075, the
# degree-7 truncation (1+t)(1+t^2)(1+t^4) of 1/(1-t) is exact to fp32
# (t^8 < 1e-9), so one 8-stage custom DVE op implements a whole sweep with
# no reciprocal anywhere.  chi(x) = 1/(1+x^3) by the same argument
# (x^3 <= 0.027, quartic-term error < 6e-7).
# ---------------------------------------------------------------------------
import numpy as _np
from concourse import dve_ops as _dvo
from concourse.dve_spec import Spec as _Spec, Src0 as _S0, Src1 as _S1, One as _One
from concourse.dve_spec import lower as _dve_lower, _has_src1
from concourse.dve_uop import DveOpSpec as _DveOpSpec


def _register_dve_op(name, spec, subdim=False):
    existing = {op.name: op for op in _dvo.OPS}
    if name in existing:
        return existing[name]
    row = max(_dvo._SUB_OPCODE_FOR_NAME.values()) + 1
    assert row < 0x20
    shas = {}
    for ver in ("v3", "v4"):
        compiled = _DveOpSpec(
            name=name, opcode=row, uops=_dve_lower(spec, ver=ver),
            rd1_en=_has_src1(spec),
        )
        shas[ver] = compiled.sha(ver)
        _dvo._COMPILE_CACHE[(name, ver)] = compiled
    op = _dvo.DveOp(name, spec, subdim=subdim, uops_sha=shas)
    _dvo.OPS.append(op)
    _dvo._SUB_OPCODE_FOR_NAME[name] = row
    _dvo.CUSTOM_DVE_SPECS[name] = spec
    return op


def _series_chi(c):
    # 1 - c + c^2 - c^3 = 1 - c*(1 - c*(1 - c))
    return _One - c * (_One - c * (_One - c))


def _ref_chi(in0, in1, c0, c1, c2):
    x = in0.astype(_np.float32)
    c = x * x * x
    one = _np.float32(1.0)
    return one - c * (one - c * (one - c))


def _telescope(t):
    # (1+t)(1+t^2)(1+t^4) = sum_{j=0..7} t^j  ~= 1/(1-t) for |t| << 1
    t2 = t * t
    t4 = t2 * t2
    return ((_One + t) * (_One + t2)) * (_One + t4)


def _ref_gs_init(in0, in1, c0, c1, c2):
    t = in0.astype(_np.float32)
    t2 = t * t
    t4 = t2 * t2
    return ((1 + t) * (1 + t2)) * (1 + t4)


def _ref_gs_step(in0, in1, c0, c1, c2):
    t = (in0.astype(_np.float32) * in1.astype(_np.float32)).astype(_np.float32)
    t2 = t * t
    t4 = t2 * t2
    return ((1 + t) * (1 + t2)) * (1 + t4)


def _ref_negmul(in0, in1, c0, c1, c2):
    return -(in0.astype(_np.float32) * in1.astype(_np.float32))


_sq0 = _S0 * _S0
OP_CHI = _register_dve_op(
    "TRIDIAG_CHI", _Spec(body=_series_chi(_sq0 * _S0), reference=_ref_chi)
)
OP_GS_INIT = _register_dve_op(
    "TRIDIAG_GS_INIT", _Spec(body=_telescope(_S0), reference=_ref_gs_init)
)
OP_GS_STEP = _register_dve_op(
    "TRIDIAG_GS_STEP", _Spec(body=_telescope(_S0 * _S1), reference=_ref_gs_step)
)
OP_NEGMUL = _register_dve_op(
    "TRIDIAG_NEGMUL", _Spec(body=-(_S0 * _S1), reference=_ref_negmul)
)


def _ref_subone_mul(in0, in1, c0, c1, c2):
    return (in0.astype(_np.float32) - _np.float32(1.0)) * in1.astype(_np.float32)


def _ref_onesub_mul(in0, in1, c0, c1, c2):
    return (_np.float32(1.0) - in0.astype(_np.float32)) * in1.astype(_np.float32)


OP_SUBONE_MUL = _register_dve_op(
    "TRIDIAG_SUBONE_MUL", _Spec(body=(_S0 - _One) * _S1, reference=_ref_subone_mul)
)
OP_ONESUB_MUL = _register_dve_op(
    "TRIDIAG_ONESUB_MUL", _Spec(body=(_One - _S0) * _S1, reference=_ref_onesub_mul)
)


def _np_chi(x, terms=3):
    c = (x * x * x).astype(_np.float32)
    one = _np.float32(1.0)
    if terms == 3:
        return one - c * (one - c * (one - c))
    return one - c * (one - c)


def _ref_phi(in0, in1, c0, c1, c2):
    x = in0.astype(_np.float32)
    return (x * x) * _np_chi(x)


def _ref_psi(in0, in1, c0, c1, c2):
    x = in0.astype(_np.float32)
    return (x * (x + _np.float32(2.0))) * _np_chi(x, terms=2)


def _ref_rchi(in0, in1, c0, c1, c2):
    x = in0.astype(_np.float32)
    return _np_chi(x) * in1.astype(_np.float32)


_TWO = _One + _One
OP_PHI = _register_dve_op(
    "TRIDIAG_PHI", _Spec(body=_series_chi(_sq0 * _S0) * _sq0, reference=_ref_phi)
)
_c_psi = _sq0 * _S0
OP_PSI = _register_dve_op(
    "TRIDIAG_PSI",
    _Spec(
        body=(_S0 * (_S0 + _TWO)) * (_One - _c_psi * (_One - _c_psi)),
        reference=_ref_psi,
    ),
)
OP_RCHI = _register_dve_op(
    "TRIDIAG_RCHI", _Spec(body=_series_chi(_sq0 * _S0) * _S1, reference=_ref_rchi)
)


def _ref_gstep1(in0, in1, c0, c1, c2):
    t = (in0.astype(_np.float32) * (1 + in1.astype(_np.float32))).astype(_np.float32)
    t2 = t * t
    return (1 + t) * (1 + t2)


_t_g1 = _S0 * (_One + _S1)
_t2_g1 = _t_g1 * _t_g1
OP_GSTEP1 = _register_dve_op(
    "TRIDIAG_GSTEP1",
    _Spec(body=(_One + _t_g1) * (_One + _t2_g1), reference=_ref_gstep1),
)


def emit_core(
    tc, alpha_in, f_in, x_out, D=D, K=K_SWEEPS, HF=HF, HB=HB, final_acc=True
):
    """Emit one core's program.

    alpha_in: dram handle [P*D + HF + HB + 2]  (alpha padded: lane p uses
              [p*D, p*D + T + 2), covering global rows g-1, g, g+1 for its
              window rows g in [p*D - HF, p*D + D + HB))
    f_in:     dram handle [P*D + HF + HB]
    x_out:    dram handle [P*D]
    """
    nc = tc.nc
    T = HF + D + HB
    TA = T + 2
    with contextlib.ExitStack() as ctx:
        pool = ctx.enter_context(tc.tile_pool(name="w", bufs=1))
        t_alpha = pool.tile([P, TA], F32, tag="alpha")  # later reused: q
        t_f = pool.tile([P, TA], F32, tag="f")          # later: B, in place
        t_sq = pool.tile([P, TA], F32, tag="sq")        # later: A, in place
        t_um = pool.tile([P, TA], F32, tag="um")        # (alpha+1)^2 - 1 via ACT
        t_ncp = pool.tile([P, TA], F32, tag="ncp")      # negcp (GPSIMD)
        t_cube = pool.tile([P, TA], F32, tag="cube")    # later: x
        t_su = pool.tile([P, TA], F32, tag="su")        # later: dp
        t_m = pool.tile([P, TA], F32, tag="m")
        t_r = pool.tile([P, TA], F32, tag="r")

        nc.sync.dma_start(t_alpha[:], bass.AP(alpha_in, 0, [[D, P], [1, TA]]))
        nc.sync.dma_start(t_f[:, 0:T], bass.AP(f_in, 0, [[D, P], [1, T]]))

        # ACT: sq = alpha^2 ; um = (alpha+1)^2 - 1 = alpha^2 + 2 alpha
        nc.scalar.activation(t_sq[:], t_alpha[:], ACTF.Square)
        nc.scalar.activation(t_um[:], t_alpha[:], ACTF.Square, bias=1.0)
        nc.scalar.activation(t_um[:], t_um[:], ACTF.Identity, bias=-1.0)
        # cube[k] = alpha[k]^3
        nc.vector.tensor_tensor(t_cube[:], t_sq[:], t_alpha[:], ALU.mult)
        # su[t] = s_t * u_{t-1} = sq[t] * um[t+1]   (row t: g = lane_base - HF + t)
        nc.vector.tensor_tensor(
            t_su[:, 0:T], t_sq[:, 0:T], t_um[:, 1 : T + 1], ALU.mult
        )
        # m^0 = b = 1 + cube[t+1]   (tensor_scalar: 2x perf mode)
        nc.vector.tensor_scalar(t_m[:, 0:T], t_cube[:, 1 : T + 1], 1.0, None, ALU.add)
        # fixed point:  m[t] = b[t] - su[t] * r[t-1],  r = 1/m
        q = t_alpha  # alpha dead after cube
        for k in range(K):
            nc.vector.reciprocal_approx_fast(out=t_r[:, 0:T], in_=t_m[:, 0:T])
            nc.vector.tensor_tensor(
                q[:, 0 : T - 1], t_su[:, 1:T], t_r[:, 0 : T - 1], ALU.mult
            )
            nc.vector.scalar_tensor_tensor(
                t_m[:, 1:T], t_cube[:, 2 : T + 1], 1.0, q[:, 0 : T - 1],
                ALU.add, ALU.subtract,
            )
        if final_acc:
            nc.vector.reciprocal_approx_accurate(
                out=t_r[:, 0:T], in_=t_m[:, 0:T], scratch=q[:, 0:T]
            )
        else:
            nc.vector.reciprocal_approx_fast(out=t_r[:, 0:T], in_=t_m[:, 0:T])

        # rn = -r  (tensor_scalar: 2x perf mode)
        rn = t_m  # m dead after final reciprocal
        nc.vector.tensor_scalar(rn[:, 0:T], t_r[:, 0:T], -1.0, None, ALU.mult)
        # negcp = -u*r = um[t+2]*rn   (GPSIMD, overlaps the dp scan)
        nc.gpsimd.tensor_tensor(
            t_ncp[:, 0:T], t_um[:, 2 : T + 2], rn[:, 0:T], ALU.mult
        )
        # A = -s*r = sq*rn ; B = f*r
        nc.vector.tensor_tensor(t_sq[:, 0:T], t_sq[:, 0:T], rn[:, 0:T], ALU.mult)
        nc.vector.tensor_tensor(t_f[:, 0:T], t_f[:, 0:T], t_r[:, 0:T], ALU.mult)
        # dp scan: dp[t] = A[t]*dp[t-1] + B[t]
        nc.vector.tensor_tensor_scan(
            t_su[:, 0:T], t_sq[:, 0:T], t_f[:, 0:T], 0.0, ALU.mult, ALU.add
        )
        # backward scan (reversed): x[t] = negcp[t]*x[t+1] + dp[t]
        nc.vector.tensor_tensor_scan(
            t_cube[:, 0:T][:, ::-1],
            t_ncp[:, 0:T][:, ::-1],
            t_su[:, 0:T][:, ::-1],
            0.0,
            ALU.mult,
            ALU.add,
        )
        nc.sync.dma_start(
            bass.AP(x_out, 0, [[D, P], [1, D]]), t_cube[:, HF : HF + D]
        )


def emit_core_g(tc, alpha_in, f_in, x_out, D=D, K=K_G, HF=HF, HB=HB):
    """g-form pipeline: custom-DVE series ops, no reciprocal instructions.

    rho_t = su_t/(b_t b_{t-1}) = (sq[t] X[t]) * (umm[t+1] X[t+1]),
    g = 1/mu via fixed point  g <- (1+t)(1+t^2)(1+t^4), t = rho*g_prev,
    r_t = 1/m_t = X[t+1] * g_t.
    """
    nc = tc.nc
    T = HF + D + HB
    TA = T + 2
    with contextlib.ExitStack() as ctx:
        pool = ctx.enter_context(tc.tile_pool(name="w", bufs=1))
        t_alpha = pool.tile([P, TA], F32, tag="alpha")
        t_f = pool.tile([P, TA], F32, tag="f")      # -> B in place
        t_sq = pool.tile([P, TA], F32, tag="sq")    # -> A in place
        t_u1 = pool.tile([P, TA], F32, tag="u1")    # (alpha+1)^2; -> negcp
        t_pp = pool.tile([P, TA], F32, tag="pp")    # -> rho in place -> x (scan2)
        t_qq = pool.tile([P, TA], F32, tag="qq")    # -> dp (scan1)
        t_g = pool.tile([P, TA], F32, tag="g")
        t_r = pool.tile([P, TA], F32, tag="r")

        # Head is column-chunked so ACT/DVE start on the first part of alpha
        # while the rest is still in flight.
        ncol = NCOL_HEAD
        if GEO_HEAD:
            w0 = TA // (2 ** ncol - 1)
            edges = [0]
            for j in range(ncol - 1):
                edges.append(edges[-1] + w0 * (2 ** j))
            edges.append(TA)
        else:
            edges = [0] + [TA * (j + 1) // ncol for j in range(ncol)]
        for j in range(ncol):
            lo, hi = edges[j], edges[j + 1]
            dma_eng = nc.sync if j % 2 == 0 else nc.scalar
            dma_eng.dma_start(
                t_alpha[:, lo:hi], bass.AP(alpha_in, lo, [[D, P], [1, hi - lo]])
            )
            # ACT: sq = alpha^2 ; u1 = (alpha+1)^2 = alpha^2 + 2 alpha + 1
            nc.scalar.activation(t_sq[:, lo:hi], t_alpha[:, lo:hi], ACTF.Square)
            nc.scalar.activation(
                t_u1[:, lo:hi], t_alpha[:, lo:hi], ACTF.Square, bias=1.0
            )
            if HEAD_V2:
                # PP = phi(alpha) = alpha^2 chi(alpha); QQ = psi(alpha)
                nc.vector._custom_dve(
                    OP_PHI, out=t_pp[:, lo:hi], in0=t_alpha[:, lo:hi]
                )
                nc.vector._custom_dve(
                    OP_PSI, out=t_qq[:, lo:hi], in0=t_alpha[:, lo:hi]
                )
            else:
                nc.vector._custom_dve(
                    OP_CHI, out=t_r[:, lo:hi], in0=t_alpha[:, lo:hi]
                )
                nc.vector.tensor_tensor(
                    t_pp[:, lo:hi], t_sq[:, lo:hi], t_r[:, lo:hi], ALU.mult
                )
                nc.vector._custom_dve(
                    OP_SUBONE_MUL, out=t_qq[:, lo:hi], in0=t_u1[:, lo:hi],
                    in1=t_r[:, lo:hi],
                )
        # rho_t = PP[t]*QQ[t+1]
        rho = t_pp[:, 0:T]
        nc.vector.tensor_tensor(rho, t_pp[:, 0:T], t_qq[:, 1 : T + 1], ALU.mult)
        # g fixed point.  Init g0 = 1 + rho (tensor_scalar: 2x perf mode);
        # each GS_STEP then applies the exact truncated map in place
        # (the in-place shifted read sees the previous sweep's values:
        # position t-1 is read two cycles before its new value lands).
        if GS_TS_INIT == "gstep1":
            # g[0] = 1 + rho[0];  g[1:] = telescope3(rho*(1+rho@-1))
            # (two map applications in one 6-stage op)
            nc.vector.tensor_scalar(
                t_g[:, 0:1], t_pp[:, 0:1], 1.0, None, ALU.add
            )
            nc.vector._custom_dve(
                OP_GSTEP1, out=t_g[:, 1:T], in0=t_pp[:, 1:T],
                in1=t_pp[:, 0 : T - 1],
            )
            nsteps = K - 2
        elif GS_TS_INIT:
            nc.vector.tensor_scalar(t_g[:, 0:T], rho, 1.0, None, ALU.add)
            nsteps = K - 1
        else:
            nc.vector._custom_dve(OP_GS_INIT, out=t_g[:, 0:T], in0=rho)
            nsteps = K - 1
        for _ in range(nsteps):
            nc.vector._custom_dve(
                OP_GS_STEP, out=t_g[:, 1:T], in0=t_pp[:, 1:T],
                in1=t_g[:, 0 : T - 1],
            )
        # f arrives late: its only consumer (B) is ~60us into the pipeline,
        # so don't let it compete with the alpha load at kernel start.
        nc.sync.dma_start(t_f[:, 0:T], bass.AP(f_in, 0, [[D, P], [1, T]]))
        # r = chi(alpha[t+1])*g
        if HEAD_V2:
            nc.vector._custom_dve(
                OP_RCHI, out=t_r[:, 0:T], in0=t_alpha[:, 1 : T + 1],
                in1=t_g[:, 0:T],
            )
        else:
            # t_r holds X = chi(alpha); write r over it (write trails read)
            nc.vector.tensor_tensor(
                t_r[:, 0:T], t_r[:, 1 : T + 1], t_g[:, 0:T], ALU.mult
            )
        # A = -sq*r (in place) ; B = f*r (in place) ; negcp = (1-u1[t+2])*r
        nc.vector._custom_dve(
            OP_NEGMUL, out=t_sq[:, 0:T], in0=t_sq[:, 0:T], in1=t_r[:, 0:T]
        )
        nc.vector.tensor_tensor(t_f[:, 0:T], t_f[:, 0:T], t_r[:, 0:T], ALU.mult)
        nc.vector._custom_dve(
            OP_ONESUB_MUL, out=t_u1[:, 0:T], in0=t_u1[:, 2 : T + 2], in1=t_r[:, 0:T]
        )
        # dp scan then reversed back-substitution scan.  The reversed scan is
        # split into column chunks (high chunk first) chained through
        # `initial`, so each chunk's output DMA overlaps the next chunk's scan.
        nc.vector.tensor_tensor_scan(
            t_qq[:, 0:T], t_sq[:, 0:T], t_f[:, 0:T], 0.0, ALU.mult, ALU.add
        )
        nchunk = SCAN2_CHUNKS
        cuts = [0] + [HF + (D * (j + 1)) // nchunk for j in range(nchunk - 1)] + [T]
        for ci in range(len(cuts) - 2, -1, -1):
            lo, hi = cuts[ci], cuts[ci + 1]
            init = 0.0 if hi == T else t_pp[:, hi : hi + 1]
            nc.vector.tensor_tensor_scan(
                t_pp[:, lo:hi][:, ::-1],
                t_u1[:, lo:hi][:, ::-1],
                t_qq[:, lo:hi][:, ::-1],
                init,
                ALU.mult,
                ALU.add,
            )
            slo, shi = max(lo, HF), min(hi, HF + D)
            nc.sync.dma_start(
                bass.AP(x_out, slo - HF, [[D, P], [1, shi - slo]]),
                t_pp[:, slo:shi],
            )


ALGO = "g"  # "g" (custom-op pipeline) or "v1" (stock-op pipeline)
HEAD_V2 = True  # PHI/PSI/RCHI fused head vs CHI+tt head
NCOL_HEAD = 3
SCAN2_CHUNKS = 4
GEO_HEAD = False
GS_TS_INIT = "gstep1"  # "gstep1" | True (ts 1+rho) | False (GS_INIT custom)


def build_nc(D=D, K=K_SWEEPS, HF=HF, HB=HB, ncores=NCORES, final_acc=True):
    C = P * D
    nc = bacc.Bacc(
        "TRN2", target_bir_lowering=False, debug=False, num_devices=ncores
    )
    alpha_in = nc.dram_tensor("alpha_in", [C + HF + HB + 2], F32, kind="ExternalInput")
    f_in = nc.dram_tensor("f_in", [C + HF + HB], F32, kind="ExternalInput")
    x_out = nc.dram_tensor("x_out", [C], F32, kind="ExternalOutput")
    with tile.TileContext(nc) as tc:
        if ALGO == "g":
            emit_core_g(tc, alpha_in, f_in, x_out, D=D, K=K_G, HF=HF, HB=HB)
        else:
            emit_core(tc, alpha_in, f_in, x_out, D=D, K=K, HF=HF, HB=HB,
                      final_acc=final_acc)
    nc.compile()
    return nc


def shard_inputs(alpha, f, D=D, HF=HF, HB=HB, ncores=NCORES):
    C = P * D
    n = ncores * C
    alpha_pad = np.zeros(n + HF + HB + 2, dtype=np.float32)
    alpha_pad[HF + 1 : HF + 1 + n] = alpha
    f_pad = np.zeros(n + HF + HB, dtype=np.float32)
    f_pad[HF : HF + n] = f
    in_maps = []
    for c in range(ncores):
        in_maps.append(
            {
                "alpha_in": np.ascontiguousarray(alpha_pad[c * C : c * C + C + HF + HB + 2]),
                "f_in": np.ascontiguousarray(f_pad[c * C : c * C + C + HF + HB]),
            }
        )
    return in_maps


_NC_CACHE = {}


def kernel(alpha: np.ndarray, f: np.ndarray, trace: bool = False, **run_kwargs):
    from concourse import bass_utils

    alpha = np.asarray(alpha, dtype=np.float32)
    f = np.asarray(f, dtype=np.float32)
    assert alpha.shape == (N,) and f.shape == (N,)
    key = (D, K_SWEEPS, HF, HB, FINAL_ACC, ALGO, K_G, HEAD_V2, NCOL_HEAD, SCAN2_CHUNKS, GS_TS_INIT, GEO_HEAD)
    if key not in _NC_CACHE:
        _NC_CACHE[key] = build_nc(K=K_SWEEPS, final_acc=FINAL_ACC)
    nc = _NC_CACHE[key]
    in_maps = shard_inputs(alpha, f)
    res = bass_utils.run_bass_kernel_spmd(
        nc, in_maps, core_ids=list(range(NCORES)), trace=trace, **run_kwargs
    )
    out = np.concatenate([res.results[c]["x_out"] for c in range(NCORES)])
    if trace:
        kernel.last_results = res
    return out


# revision 52
# speedup vs baseline: 1.0542x; 1.0077x over previous
"""Trainium2 Bass kernel: tridiagonal solve A(alpha) x = f, N = 4M, f32.

A is strictly diagonally dominant (b = 1+alpha^3 >= 1, sub = alpha^2 <= 0.09,
super = alpha^2 + 2 alpha <= 0.69 for alpha in [0, 0.3)).  All Thomas-algorithm
recurrences therefore forget their initial conditions at a geometric rate
(forward: |a/m| <= 0.097 per step, backward: |cp| <= 0.74 per step), so the
global sequential solve can be replaced by fully independent overlapping
windows: each of 8 cores x 128 lanes owns a contiguous 4096-element chunk and
computes it exactly (to fp32) using a 16-element forward warmup halo and a
64-element backward warmup halo.  No reduced system, no collectives.

The affine recurrences (forward-eliminated rhs dp, and back-substitution)
run on the DVE tensor_tensor_scan instruction (state = d0*state + d1 per
partition along the free dim; the back-substitution scan uses negative-stride
APs to run in reverse).  The nonlinear pivot recurrence, normalized as
g_t = 1/(1 - rho_t g_{t-1}) with rho_t = su_t/(b_t b_{t-1}) in [0, 0.075],
is solved by Jacobi sweeps where one custom 8-stage DVE op applies the whole
map exactly: the degree-7 truncation (1+t)(1+t^2)(1+t^4) of 1/(1-t) is
fp32-exact for t <= 0.075.  1/b = chi(alpha) is likewise an fp32-exact cubic
series.  No reciprocal/divide instructions anywhere.

Measured on trn2 (8 cores): ~82 us NEFF exec, absmax/scale error ~8.5e-7
(the fp32 reference itself is ~1.6e-7 from float64).
"""

import contextlib

import numpy as np

import concourse.bacc as bacc
import concourse.bass as bass
import concourse.mybir as mybir
import concourse.tile as tile

N = 4_194_304
NCORES = 8
P = 128
D = N // (NCORES * P)  # 4096 elements per lane
HF = 16   # forward warmup halo
HB = 64   # backward warmup halo
K_SWEEPS = 3  # fixed-point sweeps for the pivot sequence m (v1 algo)
FINAL_ACC = True  # accurate (2-op) final reciprocal vs fast (1-op)  (v1 algo)
K_G = 4   # total g-updates (init + steps) in the g-form algo
F32 = mybir.dt.float32
ALU = mybir.AluOpType
ACTF = mybir.ActivationFunctionType

# ---------------------------------------------------------------------------
# Custom DVE ops for the g-form pipeline.
#
# The normalized pivot recurrence is mu_t = 1 - rho_t / mu_{t-1} with
# rho_t = su_t / (b_t b_{t-1}) in [0, 0.075].  Iterating directly on
# g := 1/mu:  g_t = 1/(1 - rho_t g_{t-1}).  Since t = rho*g <= 0.075, the
# degree-7 truncation (1+t)(1+t^2)(1+t^4) of 1/(1-t) is exact to fp32
# (t^8 < 1e-9), so one 8-stage custom DVE op implements a whole sweep with
# no reciprocal anywhere.  chi(x) = 1/(1+x^3) by the same argument
# (x^3 <= 0.027, quartic-term error < 6e-7).
# ---------------------------------------------------------------------------
import numpy as _np
from concourse import dve_ops as _dvo
from concourse.dve_spec import Spec as _Spec, Src0 as _S0, Src1 as _S1, One as _One
from concourse.dve_spec import lower as _dve_lower, _has_src1
from concourse.dve_uop import DveOpSpec as _DveOpSpec


def _register_dve_op(name, spec, subdim=False):
    existing = {op.name: op for op in _dvo.OPS}
    if name in existing:
        return existing[name]
    row = max(_dvo._SUB_OPCODE_FOR_NAME.values()) + 1
    assert row < 0x20
    shas = {}
    for ver in ("v3", "v4"):
        compiled = _DveOpSpec(
            name=name, opcode=row, uops=_dve_lower(spec, ver=ver),
            rd1_en=_has_src1(spec),
        )
        shas[ver] = compiled.sha(ver)
        _dvo._COMPILE_CACHE[(name, ver)] = compiled
    op = _dvo.DveOp(name, spec, subdim=subdim, uops_sha=shas)
    _dvo.OPS.append(op)
    _dvo._SUB_OPCODE_FOR_NAME[name] = row
    _dvo.CUSTOM_DVE_SPECS[name] = spec
    return op


def _series_chi(c):
    # 1 - c + c^2 - c^3 = 1 - c*(1 - c*(1 - c))
    return _One - c * (_One - c * (_One - c))


def _ref_chi(in0, in1, c0, c1, c2):
    x = in0.astype(_np.float32)
    c = x * x * x
    one = _np.float32(1.0)
    return one - c * (one - c * (one - c))


def _telescope(t):
    # (1+t)(1+t^2)(1+t^4) = sum_{j=0..7} t^j  ~= 1/(1-t) for |t| << 1
    t2 = t * t
    t4 = t2 * t2
    return ((_One + t) * (_One + t2)) * (_One + t4)


def _ref_gs_init(in0, in1, c0, c1, c2):
    t = in0.astype(_np.float32)
    t2 = t * t
    t4 = t2 * t2
    return ((1 + t) * (1 + t2)) * (1 + t4)


def _ref_gs_step(in0, in1, c0, c1, c2):
    t = (in0.astype(_np.float32) * in1.astype(_np.float32)).astype(_np.float32)
    t2 = t * t
    t4 = t2 * t2
    return ((1 + t) * (1 + t2)) * (1 + t4)


def _ref_negmul(in0, in1, c0, c1, c2):
    return -(in0.astype(_np.float32) * in1.astype(_np.float32))


_sq0 = _S0 * _S0
OP_CHI = _register_dve_op(
    "TRIDIAG_CHI", _Spec(body=_series_chi(_sq0 * _S0), reference=_ref_chi)
)
OP_GS_INIT = _register_dve_op(
    "TRIDIAG_GS_INIT", _Spec(body=_telescope(_S0), reference=_ref_gs_init)
)
OP_GS_STEP = _register_dve_op(
    "TRIDIAG_GS_STEP", _Spec(body=_telescope(_S0 * _S1), reference=_ref_gs_step)
)
OP_NEGMUL = _register_dve_op(
    "TRIDIAG_NEGMUL", _Spec(body=-(_S0 * _S1), reference=_ref_negmul)
)


def _ref_subone_mul(in0, in1, c0, c1, c2):
    return (in0.astype(_np.float32) - _np.float32(1.0)) * in1.astype(_np.float32)


def _ref_onesub_mul(in0, in1, c0, c1, c2):
    return (_np.float32(1.0) - in0.astype(_np.float32)) * in1.astype(_np.float32)


OP_SUBONE_MUL = _register_dve_op(
    "TRIDIAG_SUBONE_MUL", _Spec(body=(_S0 - _One) * _S1, reference=_ref_subone_mul)
)
OP_ONESUB_MUL = _register_dve_op(
    "TRIDIAG_ONESUB_MUL", _Spec(body=(_One - _S0) * _S1, reference=_ref_onesub_mul)
)


def _np_chi(x, terms=3):
    c = (x * x * x).astype(_np.float32)
    one = _np.float32(1.0)
    if terms == 3:
        return one - c * (one - c * (one - c))
    return one - c * (one - c)


def _ref_phi(in0, in1, c0, c1, c2):
    x = in0.astype(_np.float32)
    return (x * x) * _np_chi(x)


def _ref_psi(in0, in1, c0, c1, c2):
    x = in0.astype(_np.float32)
    return (x * (x + _np.float32(2.0))) * _np_chi(x, terms=2)


def _ref_rchi(in0, in1, c0, c1, c2):
    x = in0.astype(_np.float32)
    return _np_chi(x) * in1.astype(_np.float32)


_TWO = _One + _One
OP_PHI = _register_dve_op(
    "TRIDIAG_PHI", _Spec(body=_series_chi(_sq0 * _S0) * _sq0, reference=_ref_phi)
)
_c_psi = _sq0 * _S0
OP_PSI = _register_dve_op(
    "TRIDIAG_PSI",
    _Spec(
        body=(_S0 * (_S0 + _TWO)) * (_One - _c_psi * (_One - _c_psi)),
        reference=_ref_psi,
    ),
)
OP_RCHI = _register_dve_op(
    "TRIDIAG_RCHI", _Spec(body=_series_chi(_sq0 * _S0) * _S1, reference=_ref_rchi)
)


def _ref_gstep1(in0, in1, c0, c1, c2):
    t = (in0.astype(_np.float32) * (1 + in1.astype(_np.float32))).astype(_np.float32)
    t2 = t * t
    return (1 + t) * (1 + t2)


_t_g1 = _S0 * (_One + _S1)
_t2_g1 = _t_g1 * _t_g1
OP_GSTEP1 = _register_dve_op(
    "TRIDIAG_GSTEP1",
    _Spec(body=(_One + _t_g1) * (_One + _t2_g1), reference=_ref_gstep1),
)


def emit_core(
    tc, alpha_in, f_in, x_out, D=D, K=K_SWEEPS, HF=HF, HB=HB, final_acc=True
):
    """Emit one core's program.

    alpha_in: dram handle [P*D + HF + HB + 2]  (alpha padded: lane p uses
              [p*D, p*D + T + 2), covering global rows g-1, g, g+1 for its
              window rows g in [p*D - HF, p*D + D + HB))
    f_in:     dram handle [P*D + HF + HB]
    x_out:    dram handle [P*D]
    """
    nc = tc.nc
    T = HF + D + HB
    TA = T + 2
    with contextlib.ExitStack() as ctx:
        pool = ctx.enter_context(tc.tile_pool(name="w", bufs=1))
        t_alpha = pool.tile([P, TA], F32, tag="alpha")  # later reused: q
        t_f = pool.tile([P, TA], F32, tag="f")          # later: B, in place
        t_sq = pool.tile([P, TA], F32, tag="sq")        # later: A, in place
        t_um = pool.tile([P, TA], F32, tag="um")        # (alpha+1)^2 - 1 via ACT
        t_ncp = pool.tile([P, TA], F32, tag="ncp")      # negcp (GPSIMD)
        t_cube = pool.tile([P, TA], F32, tag="cube")    # later: x
        t_su = pool.tile([P, TA], F32, tag="su")        # later: dp
        t_m = pool.tile([P, TA], F32, tag="m")
        t_r = pool.tile([P, TA], F32, tag="r")

        nc.sync.dma_start(t_alpha[:], bass.AP(alpha_in, 0, [[D, P], [1, TA]]))
        nc.sync.dma_start(t_f[:, 0:T], bass.AP(f_in, 0, [[D, P], [1, T]]))

        # ACT: sq = alpha^2 ; um = (alpha+1)^2 - 1 = alpha^2 + 2 alpha
        nc.scalar.activation(t_sq[:], t_alpha[:], ACTF.Square)
        nc.scalar.activation(t_um[:], t_alpha[:], ACTF.Square, bias=1.0)
        nc.scalar.activation(t_um[:], t_um[:], ACTF.Identity, bias=-1.0)
        # cube[k] = alpha[k]^3
        nc.vector.tensor_tensor(t_cube[:], t_sq[:], t_alpha[:], ALU.mult)
        # su[t] = s_t * u_{t-1} = sq[t] * um[t+1]   (row t: g = lane_base - HF + t)
        nc.vector.tensor_tensor(
            t_su[:, 0:T], t_sq[:, 0:T], t_um[:, 1 : T + 1], ALU.mult
        )
        # m^0 = b = 1 + cube[t+1]   (tensor_scalar: 2x perf mode)
        nc.vector.tensor_scalar(t_m[:, 0:T], t_cube[:, 1 : T + 1], 1.0, None, ALU.add)
        # fixed point:  m[t] = b[t] - su[t] * r[t-1],  r = 1/m
        q = t_alpha  # alpha dead after cube
        for k in range(K):
            nc.vector.reciprocal_approx_fast(out=t_r[:, 0:T], in_=t_m[:, 0:T])
            nc.vector.tensor_tensor(
                q[:, 0 : T - 1], t_su[:, 1:T], t_r[:, 0 : T - 1], ALU.mult
            )
            nc.vector.scalar_tensor_tensor(
                t_m[:, 1:T], t_cube[:, 2 : T + 1], 1.0, q[:, 0 : T - 1],
                ALU.add, ALU.subtract,
            )
        if final_acc:
            nc.vector.reciprocal_approx_accurate(
                out=t_r[:, 0:T], in_=t_m[:, 0:T], scratch=q[:, 0:T]
            )
        else:
            nc.vector.reciprocal_approx_fast(out=t_r[:, 0:T], in_=t_m[:, 0:T])

        # rn = -r  (tensor_scalar: 2x perf mode)
        rn = t_m  # m dead after final reciprocal
        nc.vector.tensor_scalar(rn[:, 0:T], t_r[:, 0:T], -1.0, None, ALU.mult)
        # negcp = -u*r = um[t+2]*rn   (GPSIMD, overlaps the dp scan)
        nc.gpsimd.tensor_tensor(
            t_ncp[:, 0:T], t_um[:, 2 : T + 2], rn[:, 0:T], ALU.mult
        )
        # A = -s*r = sq*rn ; B = f*r
        nc.vector.tensor_tensor(t_sq[:, 0:T], t_sq[:, 0:T], rn[:, 0:T], ALU.mult)
        nc.vector.tensor_tensor(t_f[:, 0:T], t_f[:, 0:T], t_r[:, 0:T], ALU.mult)
        # dp scan: dp[t] = A[t]*dp[t-1] + B[t]
        nc.vector.tensor_tensor_scan(
            t_su[:, 0:T], t_sq[:, 0:T], t_f[:, 0:T], 0.0, ALU.mult, ALU.add
        )
        # backward scan (reversed): x[t] = negcp[t]*x[t+1] + dp[t]
        nc.vector.tensor_tensor_scan(
            t_cube[:, 0:T][:, ::-1],
            t_ncp[:, 0:T][:, ::-1],
            t_su[:, 0:T][:, ::-1],
            0.0,
            ALU.mult,
            ALU.add,
        )
        nc.sync.dma_start(
            bass.AP(x_out, 0, [[D, P], [1, D]]), t_cube[:, HF : HF + D]
        )


def emit_core_g(tc, alpha_in, f_in, x_out, D=D, K=K_G, HF=HF, HB=HB):
    """g-form pipeline: custom-DVE series ops, no reciprocal instructions.

    rho_t = su_t/(b_t b_{t-1}) = (sq[t] X[t]) * (umm[t+1] X[t+1]),
    g = 1/mu via fixed point  g <- (1+t)(1+t^2)(1+t^4), t = rho*g_prev,
    r_t = 1/m_t = X[t+1] * g_t.
    """
    nc = tc.nc
    T = HF + D + HB
    TA = T + 2
    with contextlib.ExitStack() as ctx:
        pool = ctx.enter_context(tc.tile_pool(name="w", bufs=1))
        t_alpha = pool.tile([P, TA], F32, tag="alpha")
        t_f = pool.tile([P, TA], F32, tag="f")      # -> B in place
        t_sq = pool.tile([P, TA], F32, tag="sq")    # -> A in place
        t_u1 = pool.tile([P, TA], F32, tag="u1")    # (alpha+1)^2; -> negcp
        t_pp = pool.tile([P, TA], F32, tag="pp")    # -> rho in place -> x (scan2)
        t_qq = pool.tile([P, TA], F32, tag="qq")    # -> dp (scan1)
        t_g = pool.tile([P, TA], F32, tag="g")
        t_r = pool.tile([P, TA], F32, tag="r")

        # Head is column-chunked so ACT/DVE start on the first part of alpha
        # while the rest is still in flight.
        ncol = NCOL_HEAD
        if GEO_HEAD:
            w0 = TA // (2 ** ncol - 1)
            edges = [0]
            for j in range(ncol - 1):
                edges.append(edges[-1] + w0 * (2 ** j))
            edges.append(TA)
        else:
            edges = [0] + [TA * (j + 1) // ncol for j in range(ncol)]
        for j in range(ncol):
            lo, hi = edges[j], edges[j + 1]
            dma_eng = nc.sync if j % 2 == 0 else nc.scalar
            dma_eng.dma_start(
                t_alpha[:, lo:hi], bass.AP(alpha_in, lo, [[D, P], [1, hi - lo]])
            )
            # ACT: sq = alpha^2 ; u1 = (alpha+1)^2 = alpha^2 + 2 alpha + 1
            nc.scalar.activation(t_sq[:, lo:hi], t_alpha[:, lo:hi], ACTF.Square)
            nc.scalar.activation(
                t_u1[:, lo:hi], t_alpha[:, lo:hi], ACTF.Square, bias=1.0
            )
            if HEAD_V2:
                # PP = phi(alpha) = alpha^2 chi(alpha); QQ = psi(alpha)
                nc.vector._custom_dve(
                    OP_PHI, out=t_pp[:, lo:hi], in0=t_alpha[:, lo:hi]
                )
                nc.vector._custom_dve(
                    OP_PSI, out=t_qq[:, lo:hi], in0=t_alpha[:, lo:hi]
                )
            else:
                nc.vector._custom_dve(
                    OP_CHI, out=t_r[:, lo:hi], in0=t_alpha[:, lo:hi]
                )
                nc.vector.tensor_tensor(
                    t_pp[:, lo:hi], t_sq[:, lo:hi], t_r[:, lo:hi], ALU.mult
                )
                nc.vector._custom_dve(
                    OP_SUBONE_MUL, out=t_qq[:, lo:hi], in0=t_u1[:, lo:hi],
                    in1=t_r[:, lo:hi],
                )
        # rho_t = PP[t]*QQ[t+1]
        rho = t_pp[:, 0:T]
        nc.vector.tensor_tensor(rho, t_pp[:, 0:T], t_qq[:, 1 : T + 1], ALU.mult)
        # g fixed point.  Init g0 = 1 + rho (tensor_scalar: 2x perf mode);
        # each GS_STEP then applies the exact truncated map in place
        # (the in-place shifted read sees the previous sweep's values:
        # position t-1 is read two cycles before its new value lands).
        if GS_TS_INIT == "gstep1":
            # g[0] = 1 + rho[0];  g[1:] = telescope3(rho*(1+rho@-1))
            # (two map applications in one 6-stage op)
            nc.vector.tensor_scalar(
                t_g[:, 0:1], t_pp[:, 0:1], 1.0, None, ALU.add
            )
            nc.vector._custom_dve(
                OP_GSTEP1, out=t_g[:, 1:T], in0=t_pp[:, 1:T],
                in1=t_pp[:, 0 : T - 1],
            )
            nsteps = K - 2
        elif GS_TS_INIT:
            nc.vector.tensor_scalar(t_g[:, 0:T], rho, 1.0, None, ALU.add)
            nsteps = K - 1
        else:
            nc.vector._custom_dve(OP_GS_INIT, out=t_g[:, 0:T], in0=rho)
            nsteps = K - 1
        for _ in range(nsteps):
            nc.vector._custom_dve(
                OP_GS_STEP, out=t_g[:, 1:T], in0=t_pp[:, 1:T],
                in1=t_g[:, 0 : T - 1],
            )
        # f arrives late: its only consumer (B) is ~60us into the pipeline,
        # so don't let it compete with the alpha load at kernel start.
        nc.sync.dma_start(t_f[:, 0:T], bass.AP(f_in, 0, [[D, P], [1, T]]))
        # r = chi(alpha[t+1])*g
        if HEAD_V2:
            nc.vector._custom_dve(
                OP_RCHI, out=t_r[:, 0:T], in0=t_alpha[:, 1 : T + 1],
                in1=t_g[:, 0:T],
            )
        else:
            # t_r holds X = chi(alpha); write r over it (write trails read)
            nc.vector.tensor_tensor(
                t_r[:, 0:T], t_r[:, 1 : T + 1], t_g[:, 0:T], ALU.mult
            )
        # A = -sq*r (in place) ; B = f*r (in place) ; negcp = (1-u1[t+2])*r
        nc.vector._custom_dve(
            OP_NEGMUL, out=t_sq[:, 0:T], in0=t_sq[:, 0:T], in1=t_r[:, 0:T]
        )
        nc.vector.tensor_tensor(t_f[:, 0:T], t_f[:, 0:T], t_r[:, 0:T], ALU.mult)
        nc.vector._custom_dve(
            OP_ONESUB_MUL, out=t_u1[:, 0:T], in0=t_u1[:, 2 : T + 2], in1=t_r[:, 0:T]
        )
        # dp scan then reversed back-substitution scan.  The reversed scan is
        # split into column chunks (high chunk first) chained through
        # `initial`, so each chunk's output DMA overlaps the next chunk's scan.
        nc.vector.tensor_tensor_scan(
            t_qq[:, 0:T], t_sq[:, 0:T], t_f[:, 0:T], 0.0, ALU.mult, ALU.add
        )
        nchunk = SCAN2_CHUNKS
        if SCAN2_TAIL_SMALL:
            # processed high->low: make the last-processed (lowest) chunk small
            # so the final scan+store+drain tail is short
            w_last = D // 8
            rest = D - w_last
            cuts = [0, HF + w_last] + [
                HF + w_last + (rest * (j + 1)) // (nchunk - 1)
                for j in range(nchunk - 1)
            ]
            cuts[-1] = T
        else:
            cuts = [0] + [HF + (D * (j + 1)) // nchunk for j in range(nchunk - 1)] + [T]
        for ci in range(len(cuts) - 2, -1, -1):
            lo, hi = cuts[ci], cuts[ci + 1]
            init = 0.0 if hi == T else t_pp[:, hi : hi + 1]
            nc.vector.tensor_tensor_scan(
                t_pp[:, lo:hi][:, ::-1],
                t_u1[:, lo:hi][:, ::-1],
                t_qq[:, lo:hi][:, ::-1],
                init,
                ALU.mult,
                ALU.add,
            )
            slo, shi = max(lo, HF), min(hi, HF + D)
            nc.sync.dma_start(
                bass.AP(x_out, slo - HF, [[D, P], [1, shi - slo]]),
                t_pp[:, slo:shi],
            )


ALGO = "g"  # "g" (custom-op pipeline) or "v1" (stock-op pipeline)
HEAD_V2 = True  # PHI/PSI/RCHI fused head vs CHI+tt head
NCOL_HEAD = 3
SCAN2_CHUNKS = 4
GEO_HEAD = False
SCAN2_TAIL_SMALL = True
GS_TS_INIT = "gstep1"  # "gstep1" | True (ts 1+rho) | False (GS_INIT custom)


def build_nc(D=D, K=K_SWEEPS, HF=HF, HB=HB, ncores=NCORES, final_acc=True):
    C = P * D
    nc = bacc.Bacc(
        "TRN2", target_bir_lowering=False, debug=False, num_devices=ncores
    )
    alpha_in = nc.dram_tensor("alpha_in", [C + HF + HB + 2], F32, kind="ExternalInput")
    f_in = nc.dram_tensor("f_in", [C + HF + HB], F32, kind="ExternalInput")
    x_out = nc.dram_tensor("x_out", [C], F32, kind="ExternalOutput")
    with tile.TileContext(nc) as tc:
        if ALGO == "g":
            emit_core_g(tc, alpha_in, f_in, x_out, D=D, K=K_G, HF=HF, HB=HB)
        else:
            emit_core(tc, alpha_in, f_in, x_out, D=D, K=K, HF=HF, HB=HB,
                      final_acc=final_acc)
    nc.compile()
    return nc


def shard_inputs(alpha, f, D=D, HF=HF, HB=HB, ncores=NCORES):
    C = P * D
    n = ncores * C
    alpha_pad = np.zeros(n + HF + HB + 2, dtype=np.float32)
    alpha_pad[HF + 1 : HF + 1 + n] = alpha
    f_pad = np.zeros(n + HF + HB, dtype=np.float32)
    f_pad[HF : HF + n] = f
    in_maps = []
    for c in range(ncores):
        in_maps.append(
            {
                "alpha_in": np.ascontiguousarray(alpha_pad[c * C : c * C + C + HF + HB + 2]),
                "f_in": np.ascontiguousarray(f_pad[c * C : c * C + C + HF + HB]),
            }
        )
    return in_maps


_NC_CACHE = {}


def kernel(alpha: np.ndarray, f: np.ndarray, trace: bool = False, **run_kwargs):
    from concourse import bass_utils

    alpha = np.asarray(alpha, dtype=np.float32)
    f = np.asarray(f, dtype=np.float32)
    assert alpha.shape == (N,) and f.shape == (N,)
    key = (D, K_SWEEPS, HF, HB, FINAL_ACC, ALGO, K_G, HEAD_V2, NCOL_HEAD, SCAN2_CHUNKS, GS_TS_INIT, GEO_HEAD, SCAN2_TAIL_SMALL)
    if key not in _NC_CACHE:
        _NC_CACHE[key] = build_nc(K=K_SWEEPS, final_acc=FINAL_ACC)
    nc = _NC_CACHE[key]
    in_maps = shard_inputs(alpha, f)
    res = bass_utils.run_bass_kernel_spmd(
        nc, in_maps, core_ids=list(range(NCORES)), trace=trace, **run_kwargs
    )
    out = np.concatenate([res.results[c]["x_out"] for c in range(NCORES)])
    if trace:
        kernel.last_results = res
    return out
